# revision 69
# baseline (speedup 1.0000x reference)
"""Multi-head causal attention (RoPE, muP scale) on 8 TRN2 NeuronCores.

Sharding: core c = (b, g) with b = c // 4 (data-parallel batch), g = c % 4
(tensor-parallel head group of 4 heads).  Each core computes q/k/v
projections for its 4 heads, RoPE, causal flash-style attention in the
transposed (sT = [k, q]) orientation, and a partial output projection
o @ wo over its heads.  The host sums the 4 per-group partials per batch
(the tensor-parallel reduce) and stacks the 2 batches.

Key performance structure (~288 us vs a 361 us bf16 ancestor):
 - q/k projections run in fp8 e4m3 with DoubleRow double pumping (2x PE
   rate; 256-deep contraction per matmul).  muP's tiny logit scale makes
   the fp8 quantization noise harmless (rel_l2 ~6.4e-3 vs 4.7e-3 all-
   bf16); v and the output projection stay bf16 because their noise
   passes straight to the output.  The fp8 descale and attention scale
   are folded into the RoPE cos table.
 - x is resident bf16 [m, t] (for v) and streamed per-chunk fp8 (for
   q/k) with one-chunk prefetch; weight/x DMAs are interleaved in
   consumption order on the sync/gpsimd queues.
 - The softmax denominator never does a full second PE pass: exp tiles
   are accumulated on the DVE (bf16 adds into dacc), and one 2x512-wide
   ones-matmul per row reduces across key partitions into B67.  The
   reduce is deferred two score-blocks into the NEXT row so the in-order
   PE queue never waits on the DVE add chain.  1/den uses the fast
   custom-DVE reciprocal approximation (~5x cheaper than DVE RECIPROCAL,
   which otherwise clogs the vector queue and stalls bank reuse).
 - The causal mask is an identity-matmul accumulate of a -1e30 upper
   triangle; RoPE uses the tan formulation (sh swap on ACT, muls on DVE).
 - Attention banks: B01 = oT for both heads, B23/B45 = double-buffered
   score tiles, B67 = den reduce + early output-projection ring.  pv
   runs three blocks behind exp.
 - Output projection blocks whose pair-1 row norms have drained stream
   into pair-1's attention (every other pv slot, retires alternating
   ACT/DVE) to fill the exp-bound PE bubbles; the rest run at the end.
 - Symmetrically, the first half of pair-1's v projection streams into
   pair-0's attention (v_sb is double-buffered to allow it); proj-1
   skips the migrated accumulation groups.
 - Output DMAs issue on the sync queue only: the gpsimd queue runs
   drain/semaphore-recycle rounds near phase ends and stalls DMAs
   queued behind them.
"""

import sys

if "/opt/trn_rl_repo" not in sys.path:
    sys.path.insert(0, "/opt/trn_rl_repo")

import numpy as np

B, T, M, H, D = 2, 2048, 2048, 16, 128
N_CORES = 8
GROUPS = 4
HPG = H // GROUPS          # heads per group (4)
ROTARY_BASE = 10000.0
ATTN_SCALE = 1.0 / 128.0

P = 128                    # partitions
TC = T // 512              # 4 t-chunks of 512
MB = M // P                # 16 m-blocks
NDP = M // 256             # 8 double-pumped fp8 contraction blocks
TB = T // P                # 16 t-blocks
NQ = 512                   # q-chunk width
X8S = 16.0                 # fp8 scale on x
W8S = 128.0                # fp8 scale on wq/wk

_CACHE = {}


def _build_program():
    from concourse import bacc, tile
    import concourse.mybir as mybir

    F32 = mybir.dt.float32
    BF16 = mybir.dt.bfloat16
    AFT = mybir.ActivationFunctionType

    F8 = mybir.dt.float8e4
    F32R = mybir.dt.float32r
    DR = mybir.MatmulPerfMode.DoubleRow

    nc = bacc.Bacc("TRN2", target_bir_lowering=False, debug=False,
                   num_devices=N_CORES)

    xt_d = nc.dram_tensor("xt", [M, T], BF16, kind="ExternalInput")
    xt8_d = nc.dram_tensor("xt8", [M, T], F8, kind="ExternalInput")
    # fp8 q/k weights: [P, pair, dp-block, i, 256] with contraction pairs
    # (m, m+128) packed along i for DoubleRow double-pumping
    wq_d = nc.dram_tensor("wq", [P, 2, NDP, 2, 256], F8, kind="ExternalInput")
    wk_d = nc.dram_tensor("wk", [P, 2, NDP, 2, 256], F8, kind="ExternalInput")
    wv_d = nc.dram_tensor("wv", [P, 2, MB, 256], BF16, kind="ExternalInput")
    wo_d = nc.dram_tensor("wo", [P, HPG, M], BF16, kind="ExternalInput")
    cc_d = nc.dram_tensor("trig_cc", [P, T], BF16, kind="ExternalInput")
    tt_d = nc.dram_tensor("trig_tt", [P, T], BF16, kind="ExternalInput")
    tri_d = nc.dram_tensor("tri_neg", [P, P], BF16, kind="ExternalInput")
    ones_d = nc.dram_tensor("onesw", [P, P], BF16, kind="ExternalInput")
    id_d = nc.dram_tensor("identw", [P, P], BF16, kind="ExternalInput")
    r_d = nc.dram_tensor("r_out", [T, M], BF16, kind="ExternalOutput")

    with tile.TileContext(nc) as tc:
        with (
            tc.tile_pool(name="consts", bufs=1) as consts,
            tc.tile_pool(name="xpool", bufs=1) as xpool,
            tc.tile_pool(name="wpool", bufs=1) as wpool,
            tc.tile_pool(name="wopool", bufs=2) as wopool,
            tc.tile_pool(name="qkv", bufs=1) as qkv,
            tc.tile_pool(name="ppool", bufs=4) as ppool,
            tc.tile_pool(name="rtmp", bufs=2) as rtmp,
            tc.tile_pool(name="opool", bufs=1) as opool,
            tc.tile_pool(name="rout", bufs=4) as rout,
            tc.tile_pool(name="psum", bufs=1, space="PSUM") as psum,
        ):
            # --- load queues: alternate the two cheap DMA triggers ---
            qs = [nc.sync, nc.gpsimd]
            qi = [0]

            def ld(out, in_, q=None):
                eng = qs[qi[0] % 2] if q is None else q
                eng.dma_start(out=out, in_=in_)
                if q is None:
                    qi[0] += 1

            tri_sb = consts.tile([P, P], BF16, tag="tri")
            ld(tri_sb[:], tri_d[:])
            ones_sb = consts.tile([P, P], BF16, tag="ones")
            ld(ones_sb[:], ones_d[:])
            id_sb = consts.tile([P, P], BF16, tag="ident")
            ld(id_sb[:], id_d[:])

            # resident x^T [m, t] (bf16 for v, fp8 for q/k), one DMA per
            # 128-row m-block, interleaved with the pair-0 weight quarters
            # in consumption order
            xt_sb = xpool.tile([P, MB, T], BF16, tag="xt")
            wq_t = wpool.tile([P, NDP, 2, 256], F8, tag="wq", name="wq_sb")
            wk_t = wpool.tile([P, NDP, 2, 256], F8, tag="wk", name="wk_sb")
            wv_t = wpool.tile([P, MB, 256], BF16, tag="wv", name="wv_sb")

            # fp8 x streams per (pair, chunk) through a 2-deep ring,
            # prefetched one chunk ahead
            t8_tiles = {}

            def ensure_t8(pair, tcx, load=True):
                key = (pair, tcx)
                if key in t8_tiles:
                    return t8_tiles[key]
                t8 = xpool.tile([P, MB, NQ], F8, tag="xt8c", name="xt8c",
                                bufs=2)
                t8_tiles[key] = t8
                if load:
                    for mb in range(MB):
                        ld(t8[:, mb, :],
                           xt8_d[mb * P:(mb + 1) * P,
                                 tcx * NQ:(tcx + 1) * NQ])
                return t8

            def ld_strips(out, in_, n):
                # split one block across n queues in partition strips
                # (full-width lines kept) so the first-needed transfers
                # land n-times sooner
                step = P // n
                for s in range(n):
                    ld(out[s * step:(s + 1) * step], in_[s * step:(s + 1) * step])

            def stream_xt_only(tcx):
                for mb in range(MB):
                    ld(xt_sb[:, mb, tcx * NQ:(tcx + 1) * NQ],
                       xt_d[mb * P:(mb + 1) * P, tcx * NQ:(tcx + 1) * NQ])

            def load_pair_weights(pair):
                q = None if pair == 0 else nc.sync
                t8 = ensure_t8(0, 0, load=False) if pair == 0 else None
                for q4 in range(4):
                    sl = slice(4 * q4, 4 * q4 + 4)
                    sl8 = slice(2 * q4, 2 * q4 + 2)
                    if pair == 0:
                        # interleave in consumption order: x quarter-columns
                        # between the weight quarters (chunk-0 cols only;
                        # later chunks stream below).  The first m-block and
                        # weight quarter are striped so the opening matmuls
                        # of chunk 0 are not waiting on single-queue DMAs.
                        mbs = list(range(4 * q4, 4 * q4 + 4))
                        if q4 == 0:
                            ld_strips(xt_sb[:, mbs[0], 0:NQ],
                                      xt_d[mbs[0] * P:(mbs[0] + 1) * P,
                                           0:NQ], 4)
                            ld_strips(t8[:, mbs[0], :],
                                      xt8_d[mbs[0] * P:(mbs[0] + 1) * P,
                                            0:NQ], 2)
                            ld_strips(wv_t[:, sl, :],
                                      wv_d[:, pair, sl, :], 4)
                        else:
                            ld(xt_sb[:, mbs[0], 0:NQ],
                               xt_d[mbs[0] * P:(mbs[0] + 1) * P, 0:NQ])
                            ld(t8[:, mbs[0], :],
                               xt8_d[mbs[0] * P:(mbs[0] + 1) * P, 0:NQ])
                            ld(wv_t[:, sl, :], wv_d[:, pair, sl, :], q=q)
                        ld(xt_sb[:, mbs[1], 0:NQ],
                           xt_d[mbs[1] * P:(mbs[1] + 1) * P, 0:NQ])
                        ld(t8[:, mbs[1], :], xt8_d[mbs[1] * P:(mbs[1] + 1) * P, 0:NQ])
                        ld(wq_t[:, sl8, :, :], wq_d[:, pair, sl8, :, :], q=q)
                        ld(xt_sb[:, mbs[2], 0:NQ],
                           xt_d[mbs[2] * P:(mbs[2] + 1) * P, 0:NQ])
                        ld(t8[:, mbs[2], :], xt8_d[mbs[2] * P:(mbs[2] + 1) * P, 0:NQ])
                        ld(wk_t[:, sl8, :, :], wk_d[:, pair, sl8, :, :], q=q)
                        ld(xt_sb[:, mbs[3], 0:NQ],
                           xt_d[mbs[3] * P:(mbs[3] + 1) * P, 0:NQ])
                        ld(t8[:, mbs[3], :], xt8_d[mbs[3] * P:(mbs[3] + 1) * P, 0:NQ])
                    else:
                        ld(wv_t[:, sl, :], wv_d[:, pair, sl, :], q=q)
                        ld(wq_t[:, sl8, :, :], wq_d[:, pair, sl8, :, :], q=q)
                        ld(wk_t[:, sl8, :, :], wk_d[:, pair, sl8, :, :], q=q)

            load_pair_weights(0)
            cc_sb = consts.tile([P, T], BF16, tag="cc")
            ld(cc_sb[:], cc_d[:])
            tt_sb = consts.tile([P, T], BF16, tag="tt")
            ld(tt_sb[:], tt_d[:])

            def stream_xt(tcx):
                # bf16 x chunk for the v projection (resident across pairs),
                # interleaved with the same chunk's fp8 x in consumption
                # order — issued one chunk ahead of use
                t8 = ensure_t8(0, tcx, load=False)
                for mb in range(MB):
                    ld(xt_sb[:, mb, tcx * NQ:(tcx + 1) * NQ],
                       xt_d[mb * P:(mb + 1) * P, tcx * NQ:(tcx + 1) * NQ])
                    ld(t8[:, mb, :],
                       xt8_d[mb * P:(mb + 1) * P, tcx * NQ:(tcx + 1) * NQ])

            # oT for all 4 heads of the group: [d, h4 * T + t], bf16
            oT_sb = opool.tile([P, HPG * T], BF16, tag="oT")
            pend_norm = []   # deferred (h4, qc, denc, oc)

            # warmup: ramp the PE clock on the first-landed const tile
            # (the tri DMA completes ~5.5us in, ~2us before the memset
            # path would be ready)
            # B23 is the last bank chunk-0 needs (k matmuls), so the
            # warmup can keep the PE clock ramped there until the first
            # v matmul's data lands (~14.5us)
            ps_w = psum.tile([P, 2, NQ], F32, tag="B23", name="ps_warm")
            for wi in range(40):
                nc.tensor.matmul(ps_w[:, 1, 0:P], tri_sb[:],
                                 tri_sb[:], start=True, stop=True)

            wo_mcs = []
            pend_den = []    # (dacc, ocs, qc, pair) awaiting reduce

            # ---- early output projection: once a tb-row's pair-1 norm
            # has drained (two attention rows later), its full 4-head
            # outproj blocks stream into attn-1's PE bubbles on the B67
            # bank pair (shared ring with the denominator reduce) ----
            op_blocks = [(tb, mc) for tb in range(TB) for mc in range(4)]
            op_i = [0]
            op_bank = [None]
            opa_tick = [0]
            normed1 = [0]     # pair-1 rows fully normalized

            def emit_early_op(n):
                for _ in range(n):
                    if op_i[0] >= len(op_blocks):
                        return
                    tb, mc = op_blocks[op_i[0]]
                    if tb // 4 >= normed1[0] // 2:
                        return
                    k = op_i[0]
                    if k % 2 == 0:
                        op_bank[0] = psum.tile([P, 2, NQ], F32, tag="B67",
                                               name="opa")
                    ps_r = op_bank[0][:, k % 2, :]
                    for h4 in range(HPG):
                        nc.tensor.matmul(
                            ps_r,
                            oT_sb[:, h4 * T + tb * P:h4 * T + (tb + 1) * P],
                            wo_mcs[mc][:, h4, :],
                            start=(h4 == 0), stop=(h4 == HPG - 1))
                    ro = rout.tile([P, NQ], BF16, tag="ro", name="ro")
                    if k % 2 == 0:
                        nc.scalar.activation(ro[:], ps_r, AFT.Copy)
                    else:
                        nc.vector.tensor_scalar_add(ro[:], ps_r, 0.0)
                    nc.sync.dma_start(
                        out=r_d[tb * P:(tb + 1) * P,
                                 mc * NQ:(mc + 1) * NQ],
                        in_=ro[:])
                    op_i[0] += 1

            # pair-1's v projection can start during pair-0's attention
            # (weights + resident x are there; only v_sb needs its own
            # buffer) — groups accumulate on the B67 ring and retire on
            # the slack GPSIMD engine, filling the exp-bound PE bubbles
            v_tiles = {}

            def get_v(pair_):
                if pair_ not in v_tiles:
                    v_tiles[pair_] = qkv.tile([P, TB * 256], BF16, tag="v",
                                              name="v_sb", bufs=2)
                return v_tiles[pair_]

            ev_next = [0]      # pair-1 v groups (tcx*4+ts) emitted early
            ev_bank = [None]
            EV_CAP = 8

            def emit_early_v(n):
                for _ in range(n):
                    g = ev_next[0]
                    if g >= EV_CAP:
                        return
                    tcx_, ts_ = g // 4, g % 4
                    if g % 2 == 0:
                        ev_bank[0] = psum.tile([P, 2, NQ], F32, tag="B67",
                                               name="evb")
                    ps = ev_bank[0][:, g % 2, 0:256]
                    for mb in range(MB):
                        nc.tensor.matmul(
                            ps,
                            xt_sb[:, mb, tcx_ * NQ + ts_ * P:
                                  tcx_ * NQ + (ts_ + 1) * P],
                            wv_t[:, mb, :],
                            start=(mb == 0), stop=(mb == MB - 1))
                    tb_ = tcx_ * 4 + ts_
                    nc.vector.tensor_scalar_add(
                        get_v(1)[:, tb_ * 256:(tb_ + 1) * 256], ps, 0.0)
                    ev_next[0] += 1

            for pair in range(2):
                deferred_rope = None
                qT_sb = [qkv.tile([P, T], BF16, tag=f"qT{hl}", name=f"qT{hl}")
                         for hl in range(2)]
                kT_sb = [qkv.tile([P, T], BF16, tag=f"kT{hl}", name=f"kT{hl}")
                         for hl in range(2)]
                v_sb = get_v(pair)

                # ---- projections + RoPE, one 512-wide t-chunk at a time ----
                for tcx in range(TC):
                    t8 = ensure_t8(pair, tcx)
                    # prefetch the next chunk's x into the other buffer
                    if tcx + 1 < TC:
                        if pair == 0:
                            stream_xt(tcx + 1)
                        else:
                            ensure_t8(pair, tcx + 1)
                    elif pair == 0:
                        ensure_t8(1, 0)
                    tsl = slice(tcx * NQ, (tcx + 1) * NQ)
                    B01 = psum.tile([P, 2, NQ], F32, tag="B01", name="B01")
                    B23 = psum.tile([P, 2, NQ], F32, tag="B23", name="B23")
                    B45 = psum.tile([P, 2, NQ], F32, tag="B45", name="B45")
                    B67 = psum.tile([P, 2, NQ], F32, tag="B67", name="B67")
                    psq = [B01[:, hl, :] for hl in range(2)]
                    psk = [B23[:, hl, :] for hl in range(2)]
                    # one full PSUM bank per v accumulation group (only the
                    # first 256 columns are written — a bank holds a single
                    # accumulation group)
                    psv = [B45[:, 0, :], B45[:, 1, :],
                           B67[:, 0, :], B67[:, 1, :]]

                    # emit v matmuls ahead of q/k so the chunk opens with
                    # work whose banks freed earliest; q/k are fp8
                    # double-pumped over 256-deep contraction blocks
                    jobs = []
                    for mb in range(MB):
                        jobs.append(("v", mb))
                        if mb >= 4 and mb % 2 == 0:
                            jobs.append(("qk", (mb - 4) // 2))
                    jobs += [("qk", b) for b in range(NDP - 2, NDP)]

                    for kind, mb in jobs:
                        if kind == "v":
                            st, sp = (mb == 0), (mb == MB - 1)
                            for ts in range(4):
                                if pair == 1 and tcx * 4 + ts < ev_next[0]:
                                    continue
                                nc.tensor.matmul(
                                    psv[ts][:, 0:256],
                                    xt_sb[:, mb, tcx * NQ + ts * P:
                                          tcx * NQ + (ts + 1) * P],
                                    wv_t[:, mb, :], start=st, stop=sp)
                        else:
                            st, sp = (mb == 0), (mb == NDP - 1)
                            for hl in range(2):
                                nc.tensor.matmul(
                                    psq[hl],
                                    wq_t[:, mb, :, hl * P:(hl + 1) * P],
                                    t8[:, 2 * mb:2 * mb + 2, :],
                                    start=st, stop=sp, perf_mode=DR)
                                nc.tensor.matmul(
                                    psk[hl],
                                    wk_t[:, mb, :, hl * P:(hl + 1) * P],
                                    t8[:, 2 * mb:2 * mb + 2, :],
                                    start=st, stop=sp, perf_mode=DR)

                    # v bank drains (ACT) — free b4..b7 for the next chunk
                    hp = tc.high_priority()
                    hp.__enter__()
                    for ts in range(4):
                        tb = tcx * 4 + ts
                        if pair == 1 and tcx * 4 + ts < ev_next[0]:
                            continue
                        nc.scalar.activation(
                            v_sb[:, tb * 256:(tb + 1) * 256],
                            psv[ts][:, 0:256], AFT.Copy)

                    # RoPE.  rot_even = qe*cos - qo*sin ; rot_odd = qe*sin +
                    # qo*cos.  pab = [qe*cos ; qo*cos] in one DVE op against
                    # the duplicated-cos table — the only reader of the
                    # projection PSUM bank (emitted q0,k0,q1,k1 to match the
                    # next chunk's bank-need order).  sh = swap(pab) (ACT),
                    # then the sin products are sh * tan.
                    paks = []
                    for hl in range(2):   # ACT bf16 copies retire k banks
                        a_k = rtmp.tile([P, NQ], BF16, tag="ak", name="a_k",
                                        bufs=2)
                        nc.scalar.activation(a_k[:], psk[hl], AFT.Copy)
                        paks.append(a_k)
                    phase2 = []
                    for hl in range(2):   # DVE muls retire q banks
                        pab = rtmp.tile([P, NQ], F32, tag="pab", name="pab",
                                        bufs=2)
                        nc.vector.tensor_mul(pab[:], psq[hl], cc_sb[:, tsl])
                        phase2.append((pab, qT_sb[hl]))
                    hp.__exit__(None, None, None)
                    # For the last chunk only the PSUM-reading ops (above)
                    # run here; the sh/pcd/combine tail feeds only attention
                    # row qc=3, so it is deferred into the attention loop to
                    # keep the first exps at the head of the ACT stream.
                    if tcx < TC - 1:
                        for hl in range(2):
                            pabk = rtmp.tile([P, NQ], BF16, tag="pabk",
                                             name="pabk", bufs=2)
                            nc.vector.tensor_mul(pabk[:], paks[hl][:],
                                                 cc_sb[:, tsl])
                            phase2.append((pabk, kT_sb[hl]))
                        _emit_rope_tail(nc, rtmp, tt_sb, tsl, phase2)
                    else:
                        deferred_rope = (paks, phase2, tsl)

                    # flush the previous pair's pending denominator reduce
                    # and spread leftover deferred normalizations (previous
                    # pair's last rows) over the projection chunks
                    if pend_den:
                        flush_den(pend_norm)
                    if pend_norm:
                        _emit_norm(nc, rtmp, pend_norm.pop(0), oT_sb)

                    if pair == 0 and tcx == TC - 1:
                        load_pair_weights(1)
                        for mc in range(4):
                            wo_mc = wopool.tile([P, HPG, NQ], BF16,
                                                tag="womc", name="wo_mc",
                                                bufs=4)
                            ld(wo_mc[:], wo_d[:, :, mc * NQ:(mc + 1) * NQ],
                               q=nc.sync)
                            wo_mcs.append(wo_mc)


                # ---- attention: both heads share a paired-bank score
                # tile so one wide exp covers them; pv runs a few blocks
                # behind exp; the softmax denominator is accumulated on the
                # DVE (bf16 adds of the exp tiles into an f32 tile) and
                # reduced across key-partitions by a single fp32r
                # ones-matmul per row, so the PE pays ~1k columns per row
                # instead of a full second pass over p ----
                sT_tags = ("B23", "B45")
                sidx = 0
                def flush_den(pend_norm):
                    # the ones-matmul over the accumulated exp tile — emitted
                    # a couple of score blocks into the NEXT row so the
                    # in-order PE queue never waits on the DVE add chain
                    dacc_, ocs_, qc_, pair_ = pend_den.pop(0)
                    den_ps = psum.tile([P, 2, NQ], F32, tag="B67",
                                       name="den_ps")
                    for hl in range(2):
                        nc.tensor.matmul(den_ps[:, hl, :], ones_sb[:],
                                         dacc_[:, hl, :], start=True,
                                         stop=True)
                    for hl in range(2):
                        denc = rtmp.tile([P, NQ], F32, tag="denc",
                                         name="denc", bufs=2)
                        nc.vector.tensor_scalar_add(denc[:],
                                                    den_ps[:, hl, :], 0.0)
                        pend_norm.append((pair_ * 2 + hl, qc_, denc,
                                          ocs_[hl]))

                for qc in range(TC):
                    BA = psum.tile([P, 2, NQ], F32, tag="B01", name="BA")
                    ps_oT = [BA[:, 0, :], BA[:, 1, :]]
                    dacc = rtmp.tile([P, 2, NQ], BF16, tag="dacc",
                                     name="dacc", bufs=2)
                    jmax = 4 * qc + 3
                    pend_pv = []
                    for j in range(jmax + 1):
                        pat = j - 4 * qc
                        q0 = 128 * pat if pat >= 0 else 0
                        qs_ = slice(qc * NQ + q0, (qc + 1) * NQ)
                        st, sp = (j == 0), (j == jmax)
                        sT2 = psum.tile([P, 2, NQ], F32,
                                        tag=sT_tags[sidx % 2], name="sT2")
                        sidx += 1
                        for hl in range(2):
                            if pat >= 0:
                                nc.tensor.matmul(
                                    sT2[:, hl, q0:NQ],
                                    kT_sb[hl][:, j * P:(j + 1) * P],
                                    qT_sb[hl][:, qs_], start=True, stop=False)
                                nc.tensor.matmul(
                                    sT2[:, hl, q0:q0 + 128], id_sb[:],
                                    tri_sb[:], start=False, stop=True)
                            else:
                                nc.tensor.matmul(
                                    sT2[:, hl, q0:NQ],
                                    kT_sb[hl][:, j * P:(j + 1) * P],
                                    qT_sb[hl][:, qs_], start=True, stop=True)
                        pT2 = ppool.tile([P, 2, NQ], BF16, tag="pT",
                                         name="pT2")
                        nc.scalar.activation(
                            pT2[:, :, q0:NQ], sT2[:, :, q0:NQ], AFT.Exp)
                        if j == 0:
                            nc.vector.tensor_scalar_add(
                                dacc[:], pT2[:], 0.0)
                        else:
                            nc.vector.tensor_add(
                                dacc[:, :, q0:NQ], dacc[:, :, q0:NQ],
                                pT2[:, :, q0:NQ])
                        if j == 2 and pend_den:
                            flush_den(pend_norm)
                        pend_pv.append((j, q0, pT2, st, sp))
                        if len(pend_pv) > 2:
                            _emit_pv(nc, pend_pv.pop(0), ps_oT, v_sb)
                            opa_tick[0] += 1
                            if pair == 1:
                                emit_early_op(1)
                            if pair == 0 and opa_tick[0] % 3 == 0:
                                emit_early_v(1)
                    while pend_pv:
                        _emit_pv(nc, pend_pv.pop(0), ps_oT, v_sb)
                        opa_tick[0] += 1
                        if pair == 1:
                            emit_early_op(1)
                        if pair == 0 and opa_tick[0] % 3 == 0:
                            emit_early_v(1)
                    # drain copies (cheap, frees banks for qc+1) in the
                    # order the next accumulations need the banks back
                    ocs = []
                    for hl in range(2):
                        oc = rtmp.tile([P, NQ], BF16, tag="oc", name="oc",
                                       bufs=3)
                        nc.vector.tensor_scalar_add(oc[:], ps_oT[hl], 0.0)
                        ocs.append(oc)
                    pend_den.append((dacc, ocs, qc, pair))
                    if pair == 1:
                        emit_early_op(2)
                    elif pair == 0:
                        emit_early_v(1)
                    if qc == 1 and deferred_rope is not None:
                        paks, ph2, tsl_ = deferred_rope
                        _emit_rope_tail(nc, rtmp, tt_sb, tsl_, ph2)
                    elif qc == 2 and deferred_rope is not None:
                        paks, ph2, tsl_ = deferred_rope
                        deferred_rope = None
                        ph2 = []
                        for hl in range(2):
                            pabk = rtmp.tile([P, NQ], BF16, tag="pabk",
                                             name="pabk", bufs=2)
                            nc.vector.tensor_mul(pabk[:], paks[hl][:],
                                                 cc_sb[:, tsl_])
                            ph2.append((pabk, kT_sb[hl]))
                        _emit_rope_tail(nc, rtmp, tt_sb, tsl_, ph2)
                    # ... then the previous rows' reciprocal + normalize
                    # (drained fully — the fast reciprocal is cheap, and
                    # early-outproj eligibility wants pair-1 norms ASAP)
                    while pend_norm:
                        it = pend_norm.pop(0)
                        _emit_norm(nc, rtmp, it, oT_sb)
                        if it[0] >= 2:
                            normed1[0] += 1
            while pend_den:
                flush_den(pend_norm)
            while pend_norm:
                _emit_norm(nc, rtmp, pend_norm.pop(0), oT_sb)

            # ---- output projection: r[t, m] = sum_h oT_h.T @ wo_h for
            # the blocks not already streamed into attn-1 ----
            ridx = 0
            obanks = []
            while op_i[0] < len(op_blocks):
                tb, mc = op_blocks[op_i[0]]
                op_i[0] += 1
                wo_mc = wo_mcs[mc]
                if ridx % 8 == 0:
                    obanks = [psum.tile([P, 2, NQ], F32, tag=t, name="pr")
                              for t in ("B45", "B67", "B01", "B23")]
                ps_r = obanks[ridx % 4][:, (ridx // 4) % 2, :]
                for h4 in range(HPG):
                    nc.tensor.matmul(
                        ps_r,
                        oT_sb[:, h4 * T + tb * P:h4 * T + (tb + 1) * P],
                        wo_mc[:, h4, :],
                        start=(h4 == 0), stop=(h4 == HPG - 1))
                ro = rout.tile([P, NQ], BF16, tag="ro", name="ro")
                nc.scalar.activation(ro[:], ps_r, AFT.Copy)
                # sync only: the gpsimd queue runs drain/semaphore-clear
                # rounds in this phase and stalls DMAs queued behind them
                nc.sync.dma_start(
                    out=r_d[tb * P:(tb + 1) * P, mc * NQ:(mc + 1) * NQ],
                    in_=ro[:])
                ridx += 1

    nc.compile()
    return nc


def _emit_rope_tail(nc, rtmp, tt_sb, tsl, phase2, no_swap=False):
    import concourse.mybir as mybir

    F32 = mybir.dt.float32
    BF16 = mybir.dt.bfloat16
    AFT = mybir.ActivationFunctionType
    for pab, dst in phase2:
        pcd = rtmp.tile([P, NQ], BF16, tag="pcd", name="pcd", bufs=2)
        if no_swap:
            # read the swapped halves of pab directly in two DVE muls —
            # no ACT copy, so attention-phase exps are not pushed back
            nc.vector.tensor_mul(pcd[0:64, :], pab[64:128, :],
                                 tt_sb[64:128, tsl])
            nc.vector.tensor_mul(pcd[64:128, :], pab[0:64, :],
                                 tt_sb[0:64, tsl])
        else:
            sh = rtmp.tile([P, NQ], BF16, tag="sh", name="sh", bufs=2)
            nc.scalar.activation(sh[0:64, :], pab[64:128, :], AFT.Copy)
            nc.scalar.activation(sh[64:128, :], pab[0:64, :], AFT.Copy)
            nc.vector.tensor_mul(pcd[:], sh[:], tt_sb[:, tsl])
        nc.vector.tensor_sub(dst[0:64, tsl], pab[0:64, :], pcd[0:64, :])
        nc.vector.tensor_add(dst[64:128, tsl], pcd[64:128, :],
                             pab[64:128, :])


def _emit_pv(nc, item, ps_oT, v_sb):
    j, q0, pT2, st, sp = item
    for hl in range(2):
        nc.tensor.matmul(
            ps_oT[hl][:, q0:NQ],
            v_sb[:, j * 256 + hl * P:j * 256 + hl * P + P],
            pT2[:, hl, q0:NQ], start=st, stop=sp)


def _emit_norm(nc, rtmp, item, oT_sb):
    import concourse.mybir as mybir

    F32 = mybir.dt.float32
    h4, qc, denc, oc = item
    rec = rtmp.tile([P, NQ], F32, tag="rec", name="rec", bufs=2)
    nc.vector.reciprocal_approx_fast(rec[:], denc[:])
    nc.gpsimd.tensor_mul(
        oT_sb[:, h4 * T + qc * NQ:h4 * T + (qc + 1) * NQ], oc[:], rec[:])


def _host_constants():
    import ml_dtypes

    BF = ml_dtypes.bfloat16
    half = D // 2
    pos = np.arange(T, dtype=np.float64)[:, None]
    freqs = np.power(
        np.float64(ROTARY_BASE),
        -np.arange(half, dtype=np.float64) / np.float64(half))[None, :]
    rad = pos * freqs                               # [T, 64]
    # cos table carries the muP attention scale and the fp8 descale for
    # the q/k projections (x scaled by X8S, weights by W8S)
    desc = np.sqrt(ATTN_SCALE) / np.float64(X8S * W8S)
    cos = np.cos(rad).T * desc                      # [64, T]
    tan = np.tan(rad).T                             # [64, T] = sin/cos
    cc = np.concatenate([cos, cos], axis=0).astype(BF)
    tt = np.concatenate([tan, tan], axis=0).astype(BF)

    kk = np.arange(P)[:, None]
    qq = np.arange(P)[None, :]
    tri = np.where(kk <= qq, 0.0, -1e30).astype(BF)  # [128, 128]
    ones = np.ones((P, P), dtype=BF)
    ident = np.eye(P, dtype=np.float32).astype(BF)
    return cc, tt, tri, ones, ident


def kernel(x, wq, wk, wv, wo):
    import ml_dtypes

    BF = ml_dtypes.bfloat16

    x = np.asarray(x, dtype=np.float32)
    wq = np.asarray(wq, dtype=np.float32)
    wk = np.asarray(wk, dtype=np.float32)
    wv = np.asarray(wv, dtype=np.float32)
    wo = np.asarray(wo, dtype=np.float32)

    from concourse.bass_utils import run_bass_kernel_spmd

    if "nc" not in _CACHE:
        _CACHE["nc"] = _build_program()
    nc = _CACHE["nc"]

    cc, tt, tri, ones, ident = _host_constants()
    F8NP = ml_dtypes.float8_e4m3fn

    def w_layout(w, g):
        # w: [M, H, D] -> group slice -> [P, 2, MB, 256] bf16
        ws = w[:, g * HPG:(g + 1) * HPG, :].astype(np.float32)
        ws = ws.reshape(M, 2, 256)                    # pair-major head axis
        ws = ws.reshape(MB, P, 2, 256).transpose(1, 2, 0, 3)
        return np.ascontiguousarray(ws).astype(BF)

    def w8_layout(w, g):
        # w: [M, H, D] -> [P, pair, dp-block, i, 256] fp8 with contraction
        # pairs (m, m+128) along i for DoubleRow
        ws = (w[:, g * HPG:(g + 1) * HPG, :] * W8S).astype(np.float32)
        ws = ws.reshape(M, 2, 256)                    # pair-major head axis
        ws = ws.reshape(NDP, 2, P, 2, 256).transpose(2, 3, 0, 1, 4)
        ws = np.clip(ws, -240.0, 240.0)
        return np.ascontiguousarray(ws).astype(F8NP)

    xts = [np.ascontiguousarray(x[b].T).astype(BF) for b in range(B)]
    xt8s = [np.clip(np.ascontiguousarray(x[b].T) * X8S,
                    -240.0, 240.0).astype(F8NP) for b in range(B)]
    in_maps = []
    for c in range(N_CORES):
        b, g = divmod(c, GROUPS)
        wog = np.ascontiguousarray(
            wo[g * HPG:(g + 1) * HPG].transpose(1, 0, 2)).astype(BF)
        in_maps.append({
            "xt": xts[b],
            "xt8": xt8s[b],
            "wq": w8_layout(wq, g),
            "wk": w8_layout(wk, g),
            "wv": w_layout(wv, g),
            "wo": wog,
            "trig_cc": cc,
            "trig_tt": tt,
            "tri_neg": tri,
            "onesw": ones,
            "identw": ident,
        })

    res = run_bass_kernel_spmd(nc, in_maps, list(range(N_CORES)))

    r = np.zeros((B, T, M), dtype=np.float32)
    for c in range(N_CORES):
        b = c // GROUPS
        r[b] += np.asarray(res.results[c]["r_out"], dtype=np.float32)
    return r



# revision 70
# speedup vs baseline: 1.0248x; 1.0248x over previous
"""Multi-head causal attention (RoPE, muP scale) on 8 TRN2 NeuronCores.

Sharding: core c = (b, g) with b = c // 4 (data-parallel batch), g = c % 4
(tensor-parallel head group of 4 heads).  Each core computes q/k/v
projections for its 4 heads, RoPE, causal flash-style attention in the
transposed (sT = [k, q]) orientation, and a partial output projection
o @ wo over its heads.  The host sums the 4 per-group partials per batch
(the tensor-parallel reduce) and stacks the 2 batches.

Key performance structure (~288 us vs a 361 us bf16 ancestor):
 - q/k projections run in fp8 e4m3 with DoubleRow double pumping (2x PE
   rate; 256-deep contraction per matmul).  muP's tiny logit scale makes
   the fp8 quantization noise harmless (rel_l2 ~6.4e-3 vs 4.7e-3 all-
   bf16); v and the output projection stay bf16 because their noise
   passes straight to the output.  The fp8 descale and attention scale
   are folded into the RoPE cos table.
 - x is resident bf16 [m, t] (for v) and streamed per-chunk fp8 (for
   q/k) with one-chunk prefetch; weight/x DMAs are interleaved in
   consumption order on the sync/gpsimd queues.
 - The softmax denominator never does a full second PE pass: exp tiles
   are accumulated on the DVE (bf16 adds into dacc), and one 2x512-wide
   ones-matmul per row reduces across key partitions into B67.  The
   reduce is deferred two score-blocks into the NEXT row so the in-order
   PE queue never waits on the DVE add chain.  1/den uses the fast
   custom-DVE reciprocal approximation (~5x cheaper than DVE RECIPROCAL,
   which otherwise clogs the vector queue and stalls bank reuse).
 - The causal mask is an identity-matmul accumulate of a -1e30 upper
   triangle; RoPE uses the tan formulation (sh swap on ACT, muls on DVE).
 - Attention banks: B01 = oT for both heads, B23/B45 = double-buffered
   score tiles, B67 = den reduce + early output-projection ring.  pv
   runs three blocks behind exp.
 - Output projection blocks whose pair-1 row norms have drained stream
   into pair-1's attention (every other pv slot, retires alternating
   ACT/DVE) to fill the exp-bound PE bubbles; the rest run at the end.
 - Symmetrically, the first half of pair-1's v projection streams into
   pair-0's attention (v_sb is double-buffered to allow it); proj-1
   skips the migrated accumulation groups.
 - Output DMAs issue on the sync queue only: the gpsimd queue runs
   drain/semaphore-recycle rounds near phase ends and stalls DMAs
   queued behind them.
"""

import sys

if "/opt/trn_rl_repo" not in sys.path:
    sys.path.insert(0, "/opt/trn_rl_repo")

import numpy as np

B, T, M, H, D = 2, 2048, 2048, 16, 128
N_CORES = 8
GROUPS = 4
HPG = H // GROUPS          # heads per group (4)
ROTARY_BASE = 10000.0
ATTN_SCALE = 1.0 / 128.0

P = 128                    # partitions
TC = T // 512              # 4 t-chunks of 512
MB = M // P                # 16 m-blocks
NDP = M // 256             # 8 double-pumped fp8 contraction blocks
TB = T // P                # 16 t-blocks
NQ = 512                   # q-chunk width
X8S = 16.0                 # fp8 scale on x
W8S = 128.0                # fp8 scale on wq/wk

_CACHE = {}


def _build_program():
    from concourse import bacc, tile
    import concourse.mybir as mybir

    F32 = mybir.dt.float32
    BF16 = mybir.dt.bfloat16
    AFT = mybir.ActivationFunctionType

    F8 = mybir.dt.float8e4
    F32R = mybir.dt.float32r
    DR = mybir.MatmulPerfMode.DoubleRow

    nc = bacc.Bacc("TRN2", target_bir_lowering=False, debug=False,
                   num_devices=N_CORES)

    xt_d = nc.dram_tensor("xt", [M, T], BF16, kind="ExternalInput")
    xt8_d = nc.dram_tensor("xt8", [M, T], F8, kind="ExternalInput")
    # fp8 q/k weights: [P, pair, dp-block, i, 256] with contraction pairs
    # (m, m+128) packed along i for DoubleRow double-pumping
    wq_d = nc.dram_tensor("wq", [P, 2, NDP, 2, 256], F8, kind="ExternalInput")
    wk_d = nc.dram_tensor("wk", [P, 2, NDP, 2, 256], F8, kind="ExternalInput")
    wv_d = nc.dram_tensor("wv", [P, 2, MB, 256], BF16, kind="ExternalInput")
    wo_d = nc.dram_tensor("wo", [P, HPG, M], BF16, kind="ExternalInput")
    cc_d = nc.dram_tensor("trig_cc", [P, T], BF16, kind="ExternalInput")
    tt_d = nc.dram_tensor("trig_tt", [P, T], BF16, kind="ExternalInput")
    tri_d = nc.dram_tensor("tri_neg", [P, P], BF16, kind="ExternalInput")
    ones_d = nc.dram_tensor("onesw", [P, P], BF16, kind="ExternalInput")
    id_d = nc.dram_tensor("identw", [P, P], BF16, kind="ExternalInput")
    r_d = nc.dram_tensor("r_out", [T, M], BF16, kind="ExternalOutput")

    with tile.TileContext(nc) as tc:
        with (
            tc.tile_pool(name="consts", bufs=1) as consts,
            tc.tile_pool(name="xpool", bufs=1) as xpool,
            tc.tile_pool(name="wpool", bufs=1) as wpool,
            tc.tile_pool(name="wopool", bufs=2) as wopool,
            tc.tile_pool(name="qkv", bufs=1) as qkv,
            tc.tile_pool(name="ppool", bufs=4) as ppool,
            tc.tile_pool(name="rtmp", bufs=2) as rtmp,
            tc.tile_pool(name="opool", bufs=1) as opool,
            tc.tile_pool(name="rout", bufs=4) as rout,
            tc.tile_pool(name="psum", bufs=1, space="PSUM") as psum,
        ):
            # --- load queues: alternate the two cheap DMA triggers ---
            qs = [nc.sync, nc.gpsimd]
            qi = [0]

            def ld(out, in_, q=None):
                eng = qs[qi[0] % 2] if q is None else q
                eng.dma_start(out=out, in_=in_)
                if q is None:
                    qi[0] += 1

            tri_sb = consts.tile([P, P], BF16, tag="tri")
            ld(tri_sb[:], tri_d[:])
            ones_sb = consts.tile([P, P], BF16, tag="ones")
            ld(ones_sb[:], ones_d[:])
            id_sb = consts.tile([P, P], BF16, tag="ident")
            ld(id_sb[:], id_d[:])

            # resident x^T [m, t] (bf16 for v, fp8 for q/k), one DMA per
            # 128-row m-block, interleaved with the pair-0 weight quarters
            # in consumption order
            xt_sb = xpool.tile([P, MB, T], BF16, tag="xt")
            wq_t = wpool.tile([P, NDP, 2, 256], F8, tag="wq", name="wq_sb")
            wk_t = wpool.tile([P, NDP, 2, 256], F8, tag="wk", name="wk_sb")
            wv_t = wpool.tile([P, MB, 256], BF16, tag="wv", name="wv_sb")

            # fp8 x streams per (pair, chunk) through a 2-deep ring,
            # prefetched one chunk ahead
            t8_tiles = {}

            def ensure_t8(pair, tcx, load=True):
                key = (pair, tcx)
                if key in t8_tiles:
                    return t8_tiles[key]
                t8 = xpool.tile([P, MB, NQ], F8, tag="xt8c", name="xt8c",
                                bufs=2)
                t8_tiles[key] = t8
                if load:
                    for mb in range(MB):
                        ld(t8[:, mb, :],
                           xt8_d[mb * P:(mb + 1) * P,
                                 tcx * NQ:(tcx + 1) * NQ])
                return t8

            def ld_strips(out, in_, n):
                # split one block across n queues in partition strips
                # (full-width lines kept) so the first-needed transfers
                # land n-times sooner
                step = P // n
                for s in range(n):
                    ld(out[s * step:(s + 1) * step], in_[s * step:(s + 1) * step])

            def stream_xt_only(tcx):
                for mb in range(MB):
                    ld(xt_sb[:, mb, tcx * NQ:(tcx + 1) * NQ],
                       xt_d[mb * P:(mb + 1) * P, tcx * NQ:(tcx + 1) * NQ])

            def load_pair_weights(pair):
                q = None if pair == 0 else nc.sync
                t8 = ensure_t8(0, 0, load=False) if pair == 0 else None
                for q4 in range(4):
                    sl = slice(4 * q4, 4 * q4 + 4)
                    sl8 = slice(2 * q4, 2 * q4 + 2)
                    if pair == 0:
                        # interleave in consumption order: x quarter-columns
                        # between the weight quarters (chunk-0 cols only;
                        # later chunks stream below).  The first m-block and
                        # weight quarter are striped so the opening matmuls
                        # of chunk 0 are not waiting on single-queue DMAs.
                        mbs = list(range(4 * q4, 4 * q4 + 4))
                        if q4 == 0:
                            ld_strips(xt_sb[:, mbs[0], 0:NQ],
                                      xt_d[mbs[0] * P:(mbs[0] + 1) * P,
                                           0:NQ], 4)
                            ld_strips(t8[:, mbs[0], :],
                                      xt8_d[mbs[0] * P:(mbs[0] + 1) * P,
                                            0:NQ], 2)
                            ld_strips(wv_t[:, sl, :],
                                      wv_d[:, pair, sl, :], 4)
                        else:
                            ld(xt_sb[:, mbs[0], 0:NQ],
                               xt_d[mbs[0] * P:(mbs[0] + 1) * P, 0:NQ])
                            ld(t8[:, mbs[0], :],
                               xt8_d[mbs[0] * P:(mbs[0] + 1) * P, 0:NQ])
                            ld(wv_t[:, sl, :], wv_d[:, pair, sl, :], q=q)
                        ld(xt_sb[:, mbs[1], 0:NQ],
                           xt_d[mbs[1] * P:(mbs[1] + 1) * P, 0:NQ])
                        ld(t8[:, mbs[1], :], xt8_d[mbs[1] * P:(mbs[1] + 1) * P, 0:NQ])
                        ld(wq_t[:, sl8, :, :], wq_d[:, pair, sl8, :, :], q=q)
                        ld(xt_sb[:, mbs[2], 0:NQ],
                           xt_d[mbs[2] * P:(mbs[2] + 1) * P, 0:NQ])
                        ld(t8[:, mbs[2], :], xt8_d[mbs[2] * P:(mbs[2] + 1) * P, 0:NQ])
                        ld(wk_t[:, sl8, :, :], wk_d[:, pair, sl8, :, :], q=q)
                        ld(xt_sb[:, mbs[3], 0:NQ],
                           xt_d[mbs[3] * P:(mbs[3] + 1) * P, 0:NQ])
                        ld(t8[:, mbs[3], :], xt8_d[mbs[3] * P:(mbs[3] + 1) * P, 0:NQ])
                    else:
                        ld(wv_t[:, sl, :], wv_d[:, pair, sl, :], q=q)
                        ld(wq_t[:, sl8, :, :], wq_d[:, pair, sl8, :, :], q=q)
                        ld(wk_t[:, sl8, :, :], wk_d[:, pair, sl8, :, :], q=q)

            load_pair_weights(0)
            cc_sb = consts.tile([P, T], BF16, tag="cc")
            ld(cc_sb[:], cc_d[:])
            tt_sb = consts.tile([P, T], BF16, tag="tt")
            ld(tt_sb[:], tt_d[:])

            def stream_xt(tcx):
                # bf16 x chunk for the v projection (resident across pairs),
                # interleaved with the same chunk's fp8 x in consumption
                # order — issued one chunk ahead of use
                t8 = ensure_t8(0, tcx, load=False)
                for mb in range(MB):
                    ld(xt_sb[:, mb, tcx * NQ:(tcx + 1) * NQ],
                       xt_d[mb * P:(mb + 1) * P, tcx * NQ:(tcx + 1) * NQ])
                    ld(t8[:, mb, :],
                       xt8_d[mb * P:(mb + 1) * P, tcx * NQ:(tcx + 1) * NQ])

            # oT for all 4 heads of the group: [d, h4 * T + t], bf16
            oT_sb = opool.tile([P, HPG * T], BF16, tag="oT")
            pend_norm = []   # deferred (h4, qc, denc, oc)

            # warmup: ramp the PE clock on the first-landed const tile
            # (the tri DMA completes ~5.5us in, ~2us before the memset
            # path would be ready)
            # B23 is the last bank chunk-0 needs (k matmuls), so the
            # warmup can keep the PE clock ramped there until the first
            # v matmul's data lands (~14.5us)
            ps_w = psum.tile([P, 2, NQ], F32, tag="B23", name="ps_warm")
            for wi in range(40):
                nc.tensor.matmul(ps_w[:, 1, 0:P], tri_sb[:],
                                 tri_sb[:], start=True, stop=True)

            wo_mcs = []
            pend_den = []    # (dacc, ocs, qc, pair) awaiting reduce

            # ---- early output projection: once a tb-row's pair-1 norm
            # has drained (two attention rows later), its full 4-head
            # outproj blocks stream into attn-1's PE bubbles on the B67
            # bank pair (shared ring with the denominator reduce) ----
            op_blocks = [(tb, mc) for tb in range(TB) for mc in range(4)]
            op_i = [0]
            op_bank = [None]
            opa_tick = [0]
            normed1 = [0]     # pair-1 rows fully normalized

            def emit_early_op(n):
                for _ in range(n):
                    if op_i[0] >= len(op_blocks):
                        return
                    tb, mc = op_blocks[op_i[0]]
                    if tb // 4 >= normed1[0] // 2:
                        return
                    k = op_i[0]
                    if k % 2 == 0:
                        op_bank[0] = psum.tile([P, 2, NQ], F32, tag="B67",
                                               name="opa")
                    ps_r = op_bank[0][:, k % 2, :]
                    for h4 in range(HPG):
                        nc.tensor.matmul(
                            ps_r,
                            oT_sb[:, h4 * T + tb * P:h4 * T + (tb + 1) * P],
                            wo_mcs[mc][:, h4, :],
                            start=(h4 == 0), stop=(h4 == HPG - 1))
                    ro = rout.tile([P, NQ], BF16, tag="ro", name="ro")
                    if k % 2 == 0:
                        nc.scalar.activation(ro[:], ps_r, AFT.Copy)
                    else:
                        nc.vector.tensor_scalar_add(ro[:], ps_r, 0.0)
                    nc.sync.dma_start(
                        out=r_d[tb * P:(tb + 1) * P,
                                 mc * NQ:(mc + 1) * NQ],
                        in_=ro[:])
                    op_i[0] += 1

            # pair-1's v projection can start during pair-0's attention
            # (weights + resident x are there; only v_sb needs its own
            # buffer) — groups accumulate on the B67 ring and retire on
            # the slack GPSIMD engine, filling the exp-bound PE bubbles
            v_tiles = {}

            def get_v(pair_):
                if pair_ not in v_tiles:
                    v_tiles[pair_] = qkv.tile([P, TB * 256], BF16, tag="v",
                                              name="v_sb", bufs=2)
                return v_tiles[pair_]

            ev_next = [0]      # pair-1 v groups (tcx*4+ts) emitted early
            ev_bank = [None]
            EV_CAP = 8

            def emit_early_v(n):
                for _ in range(n):
                    g = ev_next[0]
                    if g >= EV_CAP:
                        return
                    tcx_, ts_ = g // 4, g % 4
                    if g % 2 == 0:
                        ev_bank[0] = psum.tile([P, 2, NQ], F32, tag="B67",
                                               name="evb")
                    ps = ev_bank[0][:, g % 2, 0:256]
                    for mb in range(MB):
                        nc.tensor.matmul(
                            ps,
                            xt_sb[:, mb, tcx_ * NQ + ts_ * P:
                                  tcx_ * NQ + (ts_ + 1) * P],
                            wv_t[:, mb, :],
                            start=(mb == 0), stop=(mb == MB - 1))
                    tb_ = tcx_ * 4 + ts_
                    nc.vector.tensor_scalar_add(
                        get_v(1)[:, tb_ * 256:(tb_ + 1) * 256], ps, 0.0)
                    ev_next[0] += 1

            for pair in range(2):
                deferred_rope = None
                qT_sb = [qkv.tile([P, T], BF16, tag=f"qT{hl}", name=f"qT{hl}")
                         for hl in range(2)]
                kT_sb = [qkv.tile([P, T], BF16, tag=f"kT{hl}", name=f"kT{hl}")
                         for hl in range(2)]
                v_sb = get_v(pair)

                # ---- projections + RoPE, one 512-wide t-chunk at a time ----
                for tcx in range(TC):
                    t8 = ensure_t8(pair, tcx)
                    # prefetch the next chunk's x into the other buffer
                    if tcx + 1 < TC:
                        if pair == 0:
                            stream_xt(tcx + 1)
                        else:
                            ensure_t8(pair, tcx + 1)
                    elif pair == 0:
                        ensure_t8(1, 0)
                    tsl = slice(tcx * NQ, (tcx + 1) * NQ)
                    B01 = psum.tile([P, 2, NQ], F32, tag="B01", name="B01")
                    B23 = psum.tile([P, 2, NQ], F32, tag="B23", name="B23")
                    B45 = psum.tile([P, 2, NQ], F32, tag="B45", name="B45")
                    B67 = psum.tile([P, 2, NQ], F32, tag="B67", name="B67")
                    psq = [B01[:, hl, :] for hl in range(2)]
                    psk = [B23[:, hl, :] for hl in range(2)]
                    # one full PSUM bank per v accumulation group (only the
                    # first 256 columns are written — a bank holds a single
                    # accumulation group)
                    psv = [B45[:, 0, :], B45[:, 1, :],
                           B67[:, 0, :], B67[:, 1, :]]

                    # emit v matmuls ahead of q/k so the chunk opens with
                    # work whose banks freed earliest; q/k are fp8
                    # double-pumped over 256-deep contraction blocks
                    jobs = []
                    for mb in range(MB):
                        jobs.append(("v", mb))
                        if mb >= 4 and mb % 2 == 0:
                            jobs.append(("qk", (mb - 4) // 2))
                    jobs += [("qk", b) for b in range(NDP - 2, NDP)]

                    for kind, mb in jobs:
                        if kind == "v":
                            st, sp = (mb == 0), (mb == MB - 1)
                            for ts in range(4):
                                if pair == 1 and tcx * 4 + ts < ev_next[0]:
                                    continue
                                nc.tensor.matmul(
                                    psv[ts][:, 0:256],
                                    xt_sb[:, mb, tcx * NQ + ts * P:
                                          tcx * NQ + (ts + 1) * P],
                                    wv_t[:, mb, :], start=st, stop=sp)
                        else:
                            st, sp = (mb == 0), (mb == NDP - 1)
                            for hl in range(2):
                                nc.tensor.matmul(
                                    psq[hl],
                                    wq_t[:, mb, :, hl * P:(hl + 1) * P],
                                    t8[:, 2 * mb:2 * mb + 2, :],
                                    start=st, stop=sp, perf_mode=DR)
                                nc.tensor.matmul(
                                    psk[hl],
                                    wk_t[:, mb, :, hl * P:(hl + 1) * P],
                                    t8[:, 2 * mb:2 * mb + 2, :],
                                    start=st, stop=sp, perf_mode=DR)

                    # v bank drains (ACT) — free b4..b7 for the next chunk
                    hp = tc.high_priority()
                    hp.__enter__()
                    for ts in range(4):
                        tb = tcx * 4 + ts
                        if pair == 1 and tcx * 4 + ts < ev_next[0]:
                            continue
                        nc.scalar.activation(
                            v_sb[:, tb * 256:(tb + 1) * 256],
                            psv[ts][:, 0:256], AFT.Copy)

                    # RoPE.  rot_even = qe*cos - qo*sin ; rot_odd = qe*sin +
                    # qo*cos.  pab = [qe*cos ; qo*cos] in one DVE op against
                    # the duplicated-cos table — the only reader of the
                    # projection PSUM bank (emitted q0,k0,q1,k1 to match the
                    # next chunk's bank-need order).  sh = swap(pab) (ACT),
                    # then the sin products are sh * tan.
                    paks = []
                    for hl in range(2):   # ACT bf16 copies retire k banks
                        a_k = rtmp.tile([P, NQ], BF16, tag="ak", name="a_k",
                                        bufs=2)
                        nc.scalar.activation(a_k[:], psk[hl], AFT.Copy)
                        paks.append(a_k)
                    phase2 = []
                    for hl in range(2):   # DVE muls retire q banks
                        pab = rtmp.tile([P, NQ], F32, tag="pab", name="pab",
                                        bufs=2)
                        nc.vector.tensor_mul(pab[:], psq[hl], cc_sb[:, tsl])
                        phase2.append((pab, qT_sb[hl]))
                    hp.__exit__(None, None, None)
                    # For the last chunk only the PSUM-reading ops (above)
                    # run here; the sh/pcd/combine tail feeds only attention
                    # row qc=3, so it is deferred into the attention loop to
                    # keep the first exps at the head of the ACT stream.
                    if tcx < TC - 1:
                        for hl in range(2):
                            pabk = rtmp.tile([P, NQ], BF16, tag="pabk",
                                             name="pabk", bufs=2)
                            nc.vector.tensor_mul(pabk[:], paks[hl][:],
                                                 cc_sb[:, tsl])
                            phase2.append((pabk, kT_sb[hl]))
                        _emit_rope_tail(nc, rtmp, tt_sb, tsl, phase2)
                    else:
                        deferred_rope = (paks, phase2, tsl)

                    # flush the previous pair's pending denominator reduce
                    # and spread leftover deferred normalizations (previous
                    # pair's last rows) over the projection chunks
                    if pend_den:
                        flush_den(pend_norm)
                    if pend_norm:
                        _emit_norm(nc, rtmp, pend_norm.pop(0), oT_sb)

                    if pair == 0 and tcx == TC - 1:
                        load_pair_weights(1)
                        for mc in range(4):
                            wo_mc = wopool.tile([P, HPG, NQ], BF16,
                                                tag="womc", name="wo_mc",
                                                bufs=4)
                            ld(wo_mc[:], wo_d[:, :, mc * NQ:(mc + 1) * NQ],
                               q=nc.sync)
                            wo_mcs.append(wo_mc)


                # ---- attention: both heads share a paired-bank score
                # tile so one wide exp covers them; pv runs a few blocks
                # behind exp; the softmax denominator is accumulated on the
                # DVE (bf16 adds of the exp tiles into an f32 tile) and
                # reduced across key-partitions by a single fp32r
                # ones-matmul per row, so the PE pays ~1k columns per row
                # instead of a full second pass over p ----
                sT_tags = ("B23", "B45")
                sidx = 0
                def flush_den(pend_norm):
                    # the ones-matmul over the accumulated exp tile — emitted
                    # a couple of score blocks into the NEXT row so the
                    # in-order PE queue never waits on the DVE add chain
                    dacc_, ocs_, qc_, pair_ = pend_den.pop(0)
                    den_ps = psum.tile([P, 2, NQ], F32, tag="B67",
                                       name="den_ps")
                    for hl in range(2):
                        nc.tensor.matmul(den_ps[:, hl, :], ones_sb[:],
                                         dacc_[:, hl, :], start=True,
                                         stop=True)
                    for hl in range(2):
                        denc = rtmp.tile([P, NQ], F32, tag="denc",
                                         name="denc", bufs=2)
                        nc.vector.tensor_scalar_add(denc[:],
                                                    den_ps[:, hl, :], 0.0)
                        pend_norm.append((pair_ * 2 + hl, qc_, denc,
                                          ocs_[hl]))

                for qc in range(TC):
                    BA = psum.tile([P, 2, NQ], F32, tag="B01", name="BA")
                    ps_oT = [BA[:, 0, :], BA[:, 1, :]]
                    dacc = rtmp.tile([P, 2, NQ], BF16, tag="dacc",
                                     name="dacc", bufs=2)
                    jmax = 4 * qc + 3
                    pend_pv = []
                    for j in range(jmax + 1):
                        pat = j - 4 * qc
                        q0 = 128 * pat if pat >= 0 else 0
                        qs_ = slice(qc * NQ + q0, (qc + 1) * NQ)
                        st, sp = (j == 0), (j == jmax)
                        sT2 = psum.tile([P, 2, NQ], F32,
                                        tag=sT_tags[sidx % 2], name="sT2")
                        sidx += 1
                        for hl in range(2):
                            if pat >= 0:
                                nc.tensor.matmul(
                                    sT2[:, hl, q0:NQ],
                                    kT_sb[hl][:, j * P:(j + 1) * P],
                                    qT_sb[hl][:, qs_], start=True, stop=False)
                                nc.tensor.matmul(
                                    sT2[:, hl, q0:q0 + 128], id_sb[:],
                                    tri_sb[:], start=False, stop=True)
                            else:
                                nc.tensor.matmul(
                                    sT2[:, hl, q0:NQ],
                                    kT_sb[hl][:, j * P:(j + 1) * P],
                                    qT_sb[hl][:, qs_], start=True, stop=True)
                        pT2 = ppool.tile([P, 2, NQ], BF16, tag="pT",
                                         name="pT2")
                        nc.scalar.activation(
                            pT2[:, :, q0:NQ], sT2[:, :, q0:NQ], AFT.Exp)
                        if j == 0:
                            nc.vector.tensor_scalar_add(
                                dacc[:], pT2[:], 0.0)
                        else:
                            nc.vector.tensor_add(
                                dacc[:, :, q0:NQ], dacc[:, :, q0:NQ],
                                pT2[:, :, q0:NQ])
                        if j == 2 and pend_den:
                            flush_den(pend_norm)
                        pend_pv.append((j, q0, pT2, st, sp))
                        if len(pend_pv) > 2:
                            _emit_pv(nc, pend_pv.pop(0), ps_oT, v_sb)
                            opa_tick[0] += 1
                            if pair == 1 and opa_tick[0] % 2 == 0:
                                emit_early_op(1)
                            if pair == 0 and opa_tick[0] % 3 == 0:
                                emit_early_v(1)
                    while pend_pv:
                        _emit_pv(nc, pend_pv.pop(0), ps_oT, v_sb)
                        opa_tick[0] += 1
                        if pair == 1 and opa_tick[0] % 2 == 0:
                            emit_early_op(1)
                        if pair == 0 and opa_tick[0] % 3 == 0:
                            emit_early_v(1)
                    # drain copies (cheap, frees banks for qc+1) in the
                    # order the next accumulations need the banks back
                    ocs = []
                    for hl in range(2):
                        oc = rtmp.tile([P, NQ], BF16, tag="oc", name="oc",
                                       bufs=3)
                        nc.vector.tensor_scalar_add(oc[:], ps_oT[hl], 0.0)
                        ocs.append(oc)
                    pend_den.append((dacc, ocs, qc, pair))
                    if pair == 1:
                        emit_early_op(2)
                    elif pair == 0:
                        emit_early_v(1)
                    if qc == 1 and deferred_rope is not None:
                        paks, ph2, tsl_ = deferred_rope
                        _emit_rope_tail(nc, rtmp, tt_sb, tsl_, ph2)
                    elif qc == 2 and deferred_rope is not None:
                        paks, ph2, tsl_ = deferred_rope
                        deferred_rope = None
                        ph2 = []
                        for hl in range(2):
                            pabk = rtmp.tile([P, NQ], BF16, tag="pabk",
                                             name="pabk", bufs=2)
                            nc.vector.tensor_mul(pabk[:], paks[hl][:],
                                                 cc_sb[:, tsl_])
                            ph2.append((pabk, kT_sb[hl]))
                        _emit_rope_tail(nc, rtmp, tt_sb, tsl_, ph2)
                    # ... then the previous rows' reciprocal + normalize
                    # (drained fully — the fast reciprocal is cheap, and
                    # early-outproj eligibility wants pair-1 norms ASAP)
                    while pend_norm:
                        it = pend_norm.pop(0)
                        _emit_norm(nc, rtmp, it, oT_sb)
                        if it[0] >= 2:
                            normed1[0] += 1
            while pend_den:
                flush_den(pend_norm)
            while pend_norm:
                _emit_norm(nc, rtmp, pend_norm.pop(0), oT_sb)

            # ---- output projection: r[t, m] = sum_h oT_h.T @ wo_h for
            # the blocks not already streamed into attn-1 ----
            ridx = 0
            obanks = []
            while op_i[0] < len(op_blocks):
                tb, mc = op_blocks[op_i[0]]
                op_i[0] += 1
                wo_mc = wo_mcs[mc]
                if ridx % 8 == 0:
                    obanks = [psum.tile([P, 2, NQ], F32, tag=t, name="pr")
                              for t in ("B45", "B67", "B01", "B23")]
                ps_r = obanks[ridx % 4][:, (ridx // 4) % 2, :]
                for h4 in range(HPG):
                    nc.tensor.matmul(
                        ps_r,
                        oT_sb[:, h4 * T + tb * P:h4 * T + (tb + 1) * P],
                        wo_mc[:, h4, :],
                        start=(h4 == 0), stop=(h4 == HPG - 1))
                ro = rout.tile([P, NQ], BF16, tag="ro", name="ro")
                nc.scalar.activation(ro[:], ps_r, AFT.Copy)
                # sync only: the gpsimd queue runs drain/semaphore-clear
                # rounds in this phase and stalls DMAs queued behind them
                nc.sync.dma_start(
                    out=r_d[tb * P:(tb + 1) * P, mc * NQ:(mc + 1) * NQ],
                    in_=ro[:])
                ridx += 1

    nc.compile()
    return nc


def _emit_rope_tail(nc, rtmp, tt_sb, tsl, phase2, no_swap=False):
    import concourse.mybir as mybir

    F32 = mybir.dt.float32
    BF16 = mybir.dt.bfloat16
    AFT = mybir.ActivationFunctionType
    for pab, dst in phase2:
        pcd = rtmp.tile([P, NQ], BF16, tag="pcd", name="pcd", bufs=2)
        if no_swap:
            # read the swapped halves of pab directly in two DVE muls —
            # no ACT copy, so attention-phase exps are not pushed back
            nc.vector.tensor_mul(pcd[0:64, :], pab[64:128, :],
                                 tt_sb[64:128, tsl])
            nc.vector.tensor_mul(pcd[64:128, :], pab[0:64, :],
                                 tt_sb[0:64, tsl])
        else:
            sh = rtmp.tile([P, NQ], BF16, tag="sh", name="sh", bufs=2)
            nc.scalar.activation(sh[0:64, :], pab[64:128, :], AFT.Copy)
            nc.scalar.activation(sh[64:128, :], pab[0:64, :], AFT.Copy)
            nc.vector.tensor_mul(pcd[:], sh[:], tt_sb[:, tsl])
        nc.vector.tensor_sub(dst[0:64, tsl], pab[0:64, :], pcd[0:64, :])
        nc.vector.tensor_add(dst[64:128, tsl], pcd[64:128, :],
                             pab[64:128, :])


def _emit_pv(nc, item, ps_oT, v_sb):
    j, q0, pT2, st, sp = item
    for hl in range(2):
        nc.tensor.matmul(
            ps_oT[hl][:, q0:NQ],
            v_sb[:, j * 256 + hl * P:j * 256 + hl * P + P],
            pT2[:, hl, q0:NQ], start=st, stop=sp)


def _emit_norm(nc, rtmp, item, oT_sb):
    import concourse.mybir as mybir

    F32 = mybir.dt.float32
    h4, qc, denc, oc = item
    rec = rtmp.tile([P, NQ], F32, tag="rec", name="rec", bufs=2)
    nc.vector.reciprocal_approx_fast(rec[:], denc[:])
    nc.gpsimd.tensor_mul(
        oT_sb[:, h4 * T + qc * NQ:h4 * T + (qc + 1) * NQ], oc[:], rec[:])


def _host_constants():
    import ml_dtypes

    BF = ml_dtypes.bfloat16
    half = D // 2
    pos = np.arange(T, dtype=np.float64)[:, None]
    freqs = np.power(
        np.float64(ROTARY_BASE),
        -np.arange(half, dtype=np.float64) / np.float64(half))[None, :]
    rad = pos * freqs                               # [T, 64]
    # cos table carries the muP attention scale and the fp8 descale for
    # the q/k projections (x scaled by X8S, weights by W8S)
    desc = np.sqrt(ATTN_SCALE) / np.float64(X8S * W8S)
    cos = np.cos(rad).T * desc                      # [64, T]
    tan = np.tan(rad).T                             # [64, T] = sin/cos
    cc = np.concatenate([cos, cos], axis=0).astype(BF)
    tt = np.concatenate([tan, tan], axis=0).astype(BF)

    kk = np.arange(P)[:, None]
    qq = np.arange(P)[None, :]
    tri = np.where(kk <= qq, 0.0, -1e30).astype(BF)  # [128, 128]
    ones = np.ones((P, P), dtype=BF)
    ident = np.eye(P, dtype=np.float32).astype(BF)
    return cc, tt, tri, ones, ident


def kernel(x, wq, wk, wv, wo):
    import ml_dtypes

    BF = ml_dtypes.bfloat16

    x = np.asarray(x, dtype=np.float32)
    wq = np.asarray(wq, dtype=np.float32)
    wk = np.asarray(wk, dtype=np.float32)
    wv = np.asarray(wv, dtype=np.float32)
    wo = np.asarray(wo, dtype=np.float32)

    from concourse.bass_utils import run_bass_kernel_spmd

    if "nc" not in _CACHE:
        _CACHE["nc"] = _build_program()
    nc = _CACHE["nc"]

    cc, tt, tri, ones, ident = _host_constants()
    F8NP = ml_dtypes.float8_e4m3fn

    def w_layout(w, g):
        # w: [M, H, D] -> group slice -> [P, 2, MB, 256] bf16
        ws = w[:, g * HPG:(g + 1) * HPG, :].astype(np.float32)
        ws = ws.reshape(M, 2, 256)                    # pair-major head axis
        ws = ws.reshape(MB, P, 2, 256).transpose(1, 2, 0, 3)
        return np.ascontiguousarray(ws).astype(BF)

    def w8_layout(w, g):
        # w: [M, H, D] -> [P, pair, dp-block, i, 256] fp8 with contraction
        # pairs (m, m+128) along i for DoubleRow
        ws = (w[:, g * HPG:(g + 1) * HPG, :] * W8S).astype(np.float32)
        ws = ws.reshape(M, 2, 256)                    # pair-major head axis
        ws = ws.reshape(NDP, 2, P, 2, 256).transpose(2, 3, 0, 1, 4)
        ws = np.clip(ws, -240.0, 240.0)
        return np.ascontiguousarray(ws).astype(F8NP)

    xts = [np.ascontiguousarray(x[b].T).astype(BF) for b in range(B)]
    xt8s = [np.clip(np.ascontiguousarray(x[b].T) * X8S,
                    -240.0, 240.0).astype(F8NP) for b in range(B)]
    in_maps = []
    for c in range(N_CORES):
        b, g = divmod(c, GROUPS)
        wog = np.ascontiguousarray(
            wo[g * HPG:(g + 1) * HPG].transpose(1, 0, 2)).astype(BF)
        in_maps.append({
            "xt": xts[b],
            "xt8": xt8s[b],
            "wq": w8_layout(wq, g),
            "wk": w8_layout(wk, g),
            "wv": w_layout(wv, g),
            "wo": wog,
            "trig_cc": cc,
            "trig_tt": tt,
            "tri_neg": tri,
            "onesw": ones,
            "identw": ident,
        })

    res = run_bass_kernel_spmd(nc, in_maps, list(range(N_CORES)))

    r = np.zeros((B, T, M), dtype=np.float32)
    for c in range(N_CORES):
        b = c // GROUPS
        r[b] += np.asarray(res.results[c]["r_out"], dtype=np.float32)
    return r



# revision 71
# speedup vs baseline: 1.0347x; 1.0096x over previous
"""Multi-head causal attention (RoPE, muP scale) on 8 TRN2 NeuronCores.

Sharding: core c = (b, g) with b = c // 4 (data-parallel batch), g = c % 4
(tensor-parallel head group of 4 heads).  Each core computes q/k/v
projections for its 4 heads, RoPE, causal flash-style attention in the
transposed (sT = [k, q]) orientation, and a partial output projection
o @ wo over its heads.  The host sums the 4 per-group partials per batch
(the tensor-parallel reduce) and stacks the 2 batches.

Key performance structure (~288 us vs a 361 us bf16 ancestor):
 - q/k projections run in fp8 e4m3 with DoubleRow double pumping (2x PE
   rate; 256-deep contraction per matmul).  muP's tiny logit scale makes
   the fp8 quantization noise harmless (rel_l2 ~6.4e-3 vs 4.7e-3 all-
   bf16); v and the output projection stay bf16 because their noise
   passes straight to the output.  The fp8 descale and attention scale
   are folded into the RoPE cos table.
 - x is resident bf16 [m, t] (for v) and streamed per-chunk fp8 (for
   q/k) with one-chunk prefetch; weight/x DMAs are interleaved in
   consumption order on the sync/gpsimd queues.
 - The softmax denominator never does a full second PE pass: exp tiles
   are accumulated on the DVE (bf16 adds into dacc), and one 2x512-wide
   ones-matmul per row reduces across key partitions into B67.  The
   reduce is deferred two score-blocks into the NEXT row so the in-order
   PE queue never waits on the DVE add chain.  1/den uses the fast
   custom-DVE reciprocal approximation (~5x cheaper than DVE RECIPROCAL,
   which otherwise clogs the vector queue and stalls bank reuse).
 - The causal mask is an identity-matmul accumulate of a -1e30 upper
   triangle; RoPE uses the tan formulation (sh swap on ACT, muls on DVE).
 - Attention banks: B01 = oT for both heads, B23/B45 = double-buffered
   score tiles, B67 = den reduce + early output-projection ring.  pv
   runs three blocks behind exp.
 - Output projection blocks whose pair-1 row norms have drained stream
   into pair-1's attention (every other pv slot, retires alternating
   ACT/DVE) to fill the exp-bound PE bubbles; the rest run at the end.
 - Symmetrically, the first half of pair-1's v projection streams into
   pair-0's attention (v_sb is double-buffered to allow it); proj-1
   skips the migrated accumulation groups.
 - Output DMAs issue on the sync queue only: the gpsimd queue runs
   drain/semaphore-recycle rounds near phase ends and stalls DMAs
   queued behind them.
"""

import sys

if "/opt/trn_rl_repo" not in sys.path:
    sys.path.insert(0, "/opt/trn_rl_repo")

import numpy as np

B, T, M, H, D = 2, 2048, 2048, 16, 128
N_CORES = 8
GROUPS = 4
HPG = H // GROUPS          # heads per group (4)
ROTARY_BASE = 10000.0
ATTN_SCALE = 1.0 / 128.0

P = 128                    # partitions
TC = T // 512              # 4 t-chunks of 512
MB = M // P                # 16 m-blocks
NDP = M // 256             # 8 double-pumped fp8 contraction blocks
TB = T // P                # 16 t-blocks
NQ = 512                   # q-chunk width
X8S = 16.0                 # fp8 scale on x
W8S = 128.0                # fp8 scale on wq/wk

_CACHE = {}


def _build_program():
    from concourse import bacc, tile
    import concourse.mybir as mybir

    F32 = mybir.dt.float32
    BF16 = mybir.dt.bfloat16
    AFT = mybir.ActivationFunctionType

    F8 = mybir.dt.float8e4
    F32R = mybir.dt.float32r
    DR = mybir.MatmulPerfMode.DoubleRow

    nc = bacc.Bacc("TRN2", target_bir_lowering=False, debug=False,
                   num_devices=N_CORES)

    xt_d = nc.dram_tensor("xt", [M, T], BF16, kind="ExternalInput")
    xt8_d = nc.dram_tensor("xt8", [M, T], F8, kind="ExternalInput")
    # fp8 q/k weights: [P, pair, dp-block, i, 256] with contraction pairs
    # (m, m+128) packed along i for DoubleRow double-pumping
    wq_d = nc.dram_tensor("wq", [P, 2, NDP, 2, 256], F8, kind="ExternalInput")
    wk_d = nc.dram_tensor("wk", [P, 2, NDP, 2, 256], F8, kind="ExternalInput")
    wv_d = nc.dram_tensor("wv", [P, 2, MB, 256], BF16, kind="ExternalInput")
    wo_d = nc.dram_tensor("wo", [P, HPG, M], BF16, kind="ExternalInput")
    cc_d = nc.dram_tensor("trig_cc", [P, T], BF16, kind="ExternalInput")
    tt_d = nc.dram_tensor("trig_tt", [P, T], BF16, kind="ExternalInput")
    tri_d = nc.dram_tensor("tri_neg", [P, P], BF16, kind="ExternalInput")
    ones_d = nc.dram_tensor("onesw", [P, P], BF16, kind="ExternalInput")
    id_d = nc.dram_tensor("identw", [P, P], BF16, kind="ExternalInput")
    r_d = nc.dram_tensor("r_out", [T, M], BF16, kind="ExternalOutput")

    with tile.TileContext(nc) as tc:
        with (
            tc.tile_pool(name="consts", bufs=1) as consts,
            tc.tile_pool(name="xpool", bufs=1) as xpool,
            tc.tile_pool(name="wpool", bufs=1) as wpool,
            tc.tile_pool(name="wopool", bufs=2) as wopool,
            tc.tile_pool(name="qkv", bufs=1) as qkv,
            tc.tile_pool(name="ppool", bufs=4) as ppool,
            tc.tile_pool(name="rtmp", bufs=2) as rtmp,
            tc.tile_pool(name="opool", bufs=1) as opool,
            tc.tile_pool(name="rout", bufs=4) as rout,
            tc.tile_pool(name="psum", bufs=1, space="PSUM") as psum,
        ):
            # --- load queues: alternate the two cheap DMA triggers ---
            qs = [nc.sync, nc.gpsimd]
            qi = [0]

            def ld(out, in_, q=None):
                eng = qs[qi[0] % 2] if q is None else q
                eng.dma_start(out=out, in_=in_)
                if q is None:
                    qi[0] += 1

            tri_sb = consts.tile([P, P], BF16, tag="tri")
            ld(tri_sb[:], tri_d[:])
            ones_sb = consts.tile([P, P], BF16, tag="ones")
            ld(ones_sb[:], ones_d[:])
            id_sb = consts.tile([P, P], BF16, tag="ident")
            ld(id_sb[:], id_d[:])

            # resident x^T [m, t] (bf16 for v, fp8 for q/k), one DMA per
            # 128-row m-block, interleaved with the pair-0 weight quarters
            # in consumption order
            xt_sb = xpool.tile([P, MB, T], BF16, tag="xt")
            wq_t = wpool.tile([P, NDP, 2, 256], F8, tag="wq", name="wq_sb")
            wk_t = wpool.tile([P, NDP, 2, 256], F8, tag="wk", name="wk_sb")
            wv_t = wpool.tile([P, MB, 256], BF16, tag="wv", name="wv_sb")

            # fp8 x streams per (pair, chunk) through a 2-deep ring,
            # prefetched one chunk ahead
            t8_tiles = {}

            def ensure_t8(pair, tcx, load=True):
                key = (pair, tcx)
                if key in t8_tiles:
                    return t8_tiles[key]
                t8 = xpool.tile([P, MB, NQ], F8, tag="xt8c", name="xt8c",
                                bufs=2)
                t8_tiles[key] = t8
                if load:
                    for mb in range(MB):
                        ld(t8[:, mb, :],
                           xt8_d[mb * P:(mb + 1) * P,
                                 tcx * NQ:(tcx + 1) * NQ])
                return t8

            def ld_strips(out, in_, n):
                # split one block across n queues in partition strips
                # (full-width lines kept) so the first-needed transfers
                # land n-times sooner
                step = P // n
                for s in range(n):
                    ld(out[s * step:(s + 1) * step], in_[s * step:(s + 1) * step])

            def stream_xt_only(tcx):
                for mb in range(MB):
                    ld(xt_sb[:, mb, tcx * NQ:(tcx + 1) * NQ],
                       xt_d[mb * P:(mb + 1) * P, tcx * NQ:(tcx + 1) * NQ])

            def load_pair_weights(pair):
                q = None if pair == 0 else nc.sync
                t8 = ensure_t8(0, 0, load=False) if pair == 0 else None
                for q4 in range(4):
                    sl = slice(4 * q4, 4 * q4 + 4)
                    sl8 = slice(2 * q4, 2 * q4 + 2)
                    if pair == 0:
                        # interleave in consumption order: x quarter-columns
                        # between the weight quarters (chunk-0 cols only;
                        # later chunks stream below).  The first m-block and
                        # weight quarter are striped so the opening matmuls
                        # of chunk 0 are not waiting on single-queue DMAs.
                        mbs = list(range(4 * q4, 4 * q4 + 4))
                        if q4 == 0:
                            ld_strips(xt_sb[:, mbs[0], 0:NQ],
                                      xt_d[mbs[0] * P:(mbs[0] + 1) * P,
                                           0:NQ], 4)
                            ld_strips(t8[:, mbs[0], :],
                                      xt8_d[mbs[0] * P:(mbs[0] + 1) * P,
                                            0:NQ], 2)
                            ld_strips(wv_t[:, sl, :],
                                      wv_d[:, pair, sl, :], 4)
                        else:
                            ld(xt_sb[:, mbs[0], 0:NQ],
                               xt_d[mbs[0] * P:(mbs[0] + 1) * P, 0:NQ])
                            ld(t8[:, mbs[0], :],
                               xt8_d[mbs[0] * P:(mbs[0] + 1) * P, 0:NQ])
                            ld(wv_t[:, sl, :], wv_d[:, pair, sl, :], q=q)
                        ld(xt_sb[:, mbs[1], 0:NQ],
                           xt_d[mbs[1] * P:(mbs[1] + 1) * P, 0:NQ])
                        ld(t8[:, mbs[1], :], xt8_d[mbs[1] * P:(mbs[1] + 1) * P, 0:NQ])
                        ld(wq_t[:, sl8, :, :], wq_d[:, pair, sl8, :, :], q=q)
                        ld(xt_sb[:, mbs[2], 0:NQ],
                           xt_d[mbs[2] * P:(mbs[2] + 1) * P, 0:NQ])
                        ld(t8[:, mbs[2], :], xt8_d[mbs[2] * P:(mbs[2] + 1) * P, 0:NQ])
                        ld(wk_t[:, sl8, :, :], wk_d[:, pair, sl8, :, :], q=q)
                        ld(xt_sb[:, mbs[3], 0:NQ],
                           xt_d[mbs[3] * P:(mbs[3] + 1) * P, 0:NQ])
                        ld(t8[:, mbs[3], :], xt8_d[mbs[3] * P:(mbs[3] + 1) * P, 0:NQ])
                    else:
                        ld(wv_t[:, sl, :], wv_d[:, pair, sl, :], q=q)
                        ld(wq_t[:, sl8, :, :], wq_d[:, pair, sl8, :, :], q=q)
                        ld(wk_t[:, sl8, :, :], wk_d[:, pair, sl8, :, :], q=q)

            load_pair_weights(0)
            cc_sb = consts.tile([P, T], BF16, tag="cc")
            ld(cc_sb[:], cc_d[:])
            tt_sb = consts.tile([P, T], BF16, tag="tt")
            ld(tt_sb[:], tt_d[:])

            def stream_xt(tcx):
                # bf16 x chunk for the v projection (resident across pairs),
                # interleaved with the same chunk's fp8 x in consumption
                # order — issued one chunk ahead of use
                t8 = ensure_t8(0, tcx, load=False)
                for mb in range(MB):
                    ld(xt_sb[:, mb, tcx * NQ:(tcx + 1) * NQ],
                       xt_d[mb * P:(mb + 1) * P, tcx * NQ:(tcx + 1) * NQ])
                    ld(t8[:, mb, :],
                       xt8_d[mb * P:(mb + 1) * P, tcx * NQ:(tcx + 1) * NQ])

            # oT for all 4 heads of the group: [d, h4 * T + t], bf16
            oT_sb = opool.tile([P, HPG * T], BF16, tag="oT")
            pend_norm = []   # deferred (h4, qc, denc, oc)

            # warmup: ramp the PE clock on the first-landed const tile
            # (the tri DMA completes ~5.5us in, ~2us before the memset
            # path would be ready)
            # B23 is the last bank chunk-0 needs (k matmuls), so the
            # warmup can keep the PE clock ramped there until the first
            # v matmul's data lands (~14.5us)
            ps_w = psum.tile([P, 2, NQ], F32, tag="B23", name="ps_warm")
            for wi in range(40):
                nc.tensor.matmul(ps_w[:, 1, 0:P], tri_sb[:],
                                 tri_sb[:], start=True, stop=True)

            wo_mcs = []
            pend_den = []    # (dacc, ocs, qc, pair) awaiting reduce

            # ---- early output projection: once a tb-row's pair-1 norm
            # has drained (two attention rows later), its full 4-head
            # outproj blocks stream into attn-1's PE bubbles on the B67
            # bank pair (shared ring with the denominator reduce) ----
            op_blocks = [(tb, mc) for tb in range(TB) for mc in range(4)]
            op_i = [0]
            op_bank = [None]
            opa_tick = [0]
            normed1 = [0]     # pair-1 rows fully normalized

            def emit_early_op(n):
                for _ in range(n):
                    if op_i[0] >= len(op_blocks):
                        return
                    tb, mc = op_blocks[op_i[0]]
                    if tb // 4 >= normed1[0] // 2:
                        return
                    k = op_i[0]
                    if k % 2 == 0:
                        op_bank[0] = psum.tile([P, 2, NQ], F32, tag="B67",
                                               name="opa")
                    ps_r = op_bank[0][:, k % 2, :]
                    for h4 in range(HPG):
                        nc.tensor.matmul(
                            ps_r,
                            oT_sb[:, h4 * T + tb * P:h4 * T + (tb + 1) * P],
                            wo_mcs[mc][:, h4, :],
                            start=(h4 == 0), stop=(h4 == HPG - 1))
                    ro = rout.tile([P, NQ], BF16, tag="ro", name="ro")
                    if k % 2 == 0:
                        nc.scalar.activation(ro[:], ps_r, AFT.Copy)
                    else:
                        nc.vector.tensor_scalar_add(ro[:], ps_r, 0.0)
                    nc.sync.dma_start(
                        out=r_d[tb * P:(tb + 1) * P,
                                 mc * NQ:(mc + 1) * NQ],
                        in_=ro[:])
                    op_i[0] += 1

            # pair-1's v projection can start during pair-0's attention
            # (weights + resident x are there; only v_sb needs its own
            # buffer) — groups accumulate on the B67 ring and retire on
            # the slack GPSIMD engine, filling the exp-bound PE bubbles
            v_tiles = {}

            def get_v(pair_):
                if pair_ not in v_tiles:
                    v_tiles[pair_] = qkv.tile([P, TB * 256], BF16, tag="v",
                                              name="v_sb", bufs=2)
                return v_tiles[pair_]

            ev_next = [0]      # pair-1 v groups (tcx*4+ts) emitted early
            ev_bank = [None]
            EV_CAP = 8

            def emit_early_v(n):
                for _ in range(n):
                    g = ev_next[0]
                    if g >= EV_CAP:
                        return
                    tcx_, ts_ = g // 4, g % 4
                    if g % 2 == 0:
                        ev_bank[0] = psum.tile([P, 2, NQ], F32, tag="B67",
                                               name="evb")
                    ps = ev_bank[0][:, g % 2, 0:256]
                    for mb in range(MB):
                        nc.tensor.matmul(
                            ps,
                            xt_sb[:, mb, tcx_ * NQ + ts_ * P:
                                  tcx_ * NQ + (ts_ + 1) * P],
                            wv_t[:, mb, :],
                            start=(mb == 0), stop=(mb == MB - 1))
                    tb_ = tcx_ * 4 + ts_
                    nc.vector.tensor_scalar_add(
                        get_v(1)[:, tb_ * 256:(tb_ + 1) * 256], ps, 0.0)
                    ev_next[0] += 1

            for pair in range(2):
                deferred_rope = None
                qT_sb = [qkv.tile([P, T], BF16, tag=f"qT{hl}", name=f"qT{hl}")
                         for hl in range(2)]
                kT_sb = [qkv.tile([P, T], BF16, tag=f"kT{hl}", name=f"kT{hl}")
                         for hl in range(2)]
                v_sb = get_v(pair)

                # ---- projections + RoPE, one 512-wide t-chunk at a time ----
                for tcx in range(TC):
                    t8 = ensure_t8(pair, tcx)
                    # prefetch the next chunk's x into the other buffer
                    if tcx + 1 < TC:
                        if pair == 0:
                            stream_xt(tcx + 1)
                        else:
                            ensure_t8(pair, tcx + 1)
                    elif pair == 0:
                        ensure_t8(1, 0)
                    tsl = slice(tcx * NQ, (tcx + 1) * NQ)
                    B01 = psum.tile([P, 2, NQ], F32, tag="B01", name="B01")
                    B23 = psum.tile([P, 2, NQ], F32, tag="B23", name="B23")
                    B45 = psum.tile([P, 2, NQ], F32, tag="B45", name="B45")
                    B67 = psum.tile([P, 2, NQ], F32, tag="B67", name="B67")
                    psq = [B01[:, hl, :] for hl in range(2)]
                    psk = [B23[:, hl, :] for hl in range(2)]
                    # one full PSUM bank per v accumulation group (only the
                    # first 256 columns are written — a bank holds a single
                    # accumulation group)
                    psv = [B45[:, 0, :], B45[:, 1, :],
                           B67[:, 0, :], B67[:, 1, :]]

                    # emit v matmuls ahead of q/k so the chunk opens with
                    # work whose banks freed earliest; q/k are fp8
                    # double-pumped over 256-deep contraction blocks
                    jobs = []
                    for mb in range(MB):
                        jobs.append(("v", mb))
                        if mb >= 4 and mb % 2 == 0:
                            jobs.append(("qk", (mb - 4) // 2))
                    jobs += [("qk", b) for b in range(NDP - 2, NDP)]

                    for kind, mb in jobs:
                        if kind == "v":
                            st, sp = (mb == 0), (mb == MB - 1)
                            for ts in range(4):
                                if pair == 1 and tcx * 4 + ts < ev_next[0]:
                                    continue
                                nc.tensor.matmul(
                                    psv[ts][:, 0:256],
                                    xt_sb[:, mb, tcx * NQ + ts * P:
                                          tcx * NQ + (ts + 1) * P],
                                    wv_t[:, mb, :], start=st, stop=sp)
                        else:
                            st, sp = (mb == 0), (mb == NDP - 1)
                            for hl in range(2):
                                nc.tensor.matmul(
                                    psq[hl],
                                    wq_t[:, mb, :, hl * P:(hl + 1) * P],
                                    t8[:, 2 * mb:2 * mb + 2, :],
                                    start=st, stop=sp, perf_mode=DR)
                                nc.tensor.matmul(
                                    psk[hl],
                                    wk_t[:, mb, :, hl * P:(hl + 1) * P],
                                    t8[:, 2 * mb:2 * mb + 2, :],
                                    start=st, stop=sp, perf_mode=DR)

                    # v bank drains (ACT) — free b4..b7 for the next chunk
                    hp = tc.high_priority()
                    hp.__enter__()
                    for ts in range(4):
                        tb = tcx * 4 + ts
                        if pair == 1 and tcx * 4 + ts < ev_next[0]:
                            continue
                        nc.scalar.activation(
                            v_sb[:, tb * 256:(tb + 1) * 256],
                            psv[ts][:, 0:256], AFT.Copy)

                    # RoPE.  rot_even = qe*cos - qo*sin ; rot_odd = qe*sin +
                    # qo*cos.  pab = [qe*cos ; qo*cos] in one DVE op against
                    # the duplicated-cos table — the only reader of the
                    # projection PSUM bank (emitted q0,k0,q1,k1 to match the
                    # next chunk's bank-need order).  sh = swap(pab) (ACT),
                    # then the sin products are sh * tan.
                    paks = []
                    for hl in range(2):   # ACT bf16 copies retire k banks
                        a_k = rtmp.tile([P, NQ], BF16, tag="ak", name="a_k",
                                        bufs=2)
                        nc.scalar.activation(a_k[:], psk[hl], AFT.Copy)
                        paks.append(a_k)
                    phase2 = []
                    for hl in range(2):   # DVE muls retire q banks
                        pab = rtmp.tile([P, NQ], F32, tag="pab", name="pab",
                                        bufs=2)
                        nc.vector.tensor_mul(pab[:], psq[hl], cc_sb[:, tsl])
                        phase2.append((pab, qT_sb[hl]))
                    hp.__exit__(None, None, None)
                    # For the last chunk only the PSUM-reading ops (above)
                    # run here; the sh/pcd/combine tail feeds only attention
                    # row qc=3, so it is deferred into the attention loop to
                    # keep the first exps at the head of the ACT stream.
                    if tcx < TC - 1:
                        for hl in range(2):
                            pabk = rtmp.tile([P, NQ], BF16, tag="pabk",
                                             name="pabk", bufs=2)
                            nc.vector.tensor_mul(pabk[:], paks[hl][:],
                                                 cc_sb[:, tsl])
                            phase2.append((pabk, kT_sb[hl]))
                        _emit_rope_tail(nc, rtmp, tt_sb, tsl, phase2)
                    else:
                        deferred_rope = (paks, phase2, tsl)

                    # flush the previous pair's pending denominator reduce
                    # and spread leftover deferred normalizations (previous
                    # pair's last rows) over the projection chunks
                    if pend_den:
                        flush_den(pend_norm)
                    if pend_norm:
                        _emit_norm(nc, rtmp, pend_norm.pop(0), oT_sb)

                    if pair == 0 and tcx == TC - 1:
                        load_pair_weights(1)
                        for mc in range(4):
                            wo_mc = wopool.tile([P, HPG, NQ], BF16,
                                                tag="womc", name="wo_mc",
                                                bufs=4)
                            ld(wo_mc[:], wo_d[:, :, mc * NQ:(mc + 1) * NQ],
                               q=nc.sync)
                            wo_mcs.append(wo_mc)


                # ---- attention: both heads share a paired-bank score
                # tile so one wide exp covers them; pv runs a few blocks
                # behind exp; the softmax denominator is accumulated on the
                # DVE (bf16 adds of the exp tiles into an f32 tile) and
                # reduced across key-partitions by a single fp32r
                # ones-matmul per row, so the PE pays ~1k columns per row
                # instead of a full second pass over p ----
                sT_tags = ("B23", "B45")
                sidx = 0
                def flush_den(pend_norm):
                    # the ones-matmul over the accumulated exp tile — emitted
                    # a couple of score blocks into the NEXT row so the
                    # in-order PE queue never waits on the DVE add chain
                    dacc_, ocs_, qc_, pair_ = pend_den.pop(0)
                    den_ps = psum.tile([P, 2, NQ], F32, tag="B67",
                                       name="den_ps")
                    for hl in range(2):
                        nc.tensor.matmul(den_ps[:, hl, :], ones_sb[:],
                                         dacc_[:, hl, :], start=True,
                                         stop=True)
                    for hl in range(2):
                        denc = rtmp.tile([P, NQ], F32, tag="denc",
                                         name="denc", bufs=2)
                        nc.vector.tensor_scalar_add(denc[:],
                                                    den_ps[:, hl, :], 0.0)
                        pend_norm.append((pair_ * 2 + hl, qc_, denc,
                                          ocs_[hl]))

                for qc in range(TC):
                    BA = psum.tile([P, 2, NQ], F32, tag="B01", name="BA")
                    ps_oT = [BA[:, 0, :], BA[:, 1, :]]
                    dacc = rtmp.tile([P, 2, NQ], BF16, tag="dacc",
                                     name="dacc", bufs=2)
                    jmax = 4 * qc + 3
                    pend_pv = []
                    for j in range(jmax + 1):
                        pat = j - 4 * qc
                        q0 = 128 * pat if pat >= 0 else 0
                        qs_ = slice(qc * NQ + q0, (qc + 1) * NQ)
                        st, sp = (j == 0), (j == jmax)
                        sT2 = psum.tile([P, 2, NQ], F32,
                                        tag=sT_tags[sidx % 2], name="sT2")
                        sidx += 1
                        for hl in range(2):
                            if pat >= 0:
                                nc.tensor.matmul(
                                    sT2[:, hl, q0:NQ],
                                    kT_sb[hl][:, j * P:(j + 1) * P],
                                    qT_sb[hl][:, qs_], start=True, stop=False)
                                nc.tensor.matmul(
                                    sT2[:, hl, q0:q0 + 128], id_sb[:],
                                    tri_sb[:], start=False, stop=True)
                            else:
                                nc.tensor.matmul(
                                    sT2[:, hl, q0:NQ],
                                    kT_sb[hl][:, j * P:(j + 1) * P],
                                    qT_sb[hl][:, qs_], start=True, stop=True)
                        pT2 = ppool.tile([P, 2, NQ], BF16, tag="pT",
                                         name="pT2")
                        nc.scalar.activation(
                            pT2[:, :, q0:NQ], sT2[:, :, q0:NQ], AFT.Exp)
                        if j == 0:
                            nc.vector.tensor_scalar_add(
                                dacc[:], pT2[:], 0.0)
                        else:
                            nc.vector.tensor_add(
                                dacc[:, :, q0:NQ], dacc[:, :, q0:NQ],
                                pT2[:, :, q0:NQ])
                        if j == 2 and pend_den:
                            flush_den(pend_norm)
                        pend_pv.append((j, q0, pT2, st, sp))
                        if len(pend_pv) > 2:
                            _emit_pv(nc, pend_pv.pop(0), ps_oT, v_sb)
                            opa_tick[0] += 1
                            if pair == 1 and opa_tick[0] % 2 == 0:
                                emit_early_op(1)
                            if pair == 0 and opa_tick[0] % 3 == 0:
                                emit_early_v(1)
                    while pend_pv:
                        _emit_pv(nc, pend_pv.pop(0), ps_oT, v_sb)
                        opa_tick[0] += 1
                        if pair == 1 and opa_tick[0] % 2 == 0:
                            emit_early_op(1)
                        if pair == 0 and opa_tick[0] % 3 == 0:
                            emit_early_v(1)
                    # drain copies (cheap, frees banks for qc+1) in the
                    # order the next accumulations need the banks back
                    ocs = []
                    for hl in range(2):
                        oc = rtmp.tile([P, NQ], BF16, tag="oc", name="oc",
                                       bufs=3)
                        nc.vector.tensor_scalar_add(oc[:], ps_oT[hl], 0.0)
                        ocs.append(oc)
                    pend_den.append((dacc, ocs, qc, pair))
                    if pair == 1:
                        emit_early_op(2)
                    elif pair == 0:
                        emit_early_v(1)
                    if qc == 1 and deferred_rope is not None:
                        paks, ph2, tsl_ = deferred_rope
                        _emit_rope_tail(nc, rtmp, tt_sb, tsl_, ph2)
                    elif qc == 2 and deferred_rope is not None:
                        paks, ph2, tsl_ = deferred_rope
                        deferred_rope = None
                        ph2 = []
                        for hl in range(2):
                            pabk = rtmp.tile([P, NQ], BF16, tag="pabk",
                                             name="pabk", bufs=2)
                            nc.vector.tensor_mul(pabk[:], paks[hl][:],
                                                 cc_sb[:, tsl_])
                            ph2.append((pabk, kT_sb[hl]))
                        _emit_rope_tail(nc, rtmp, tt_sb, tsl_, ph2)
                    # ... then the previous rows' reciprocal + normalize
                    # (drained fully — the fast reciprocal is cheap, and
                    # early-outproj eligibility wants pair-1 norms ASAP)
                    while pend_norm:
                        it = pend_norm.pop(0)
                        _emit_norm(nc, rtmp, it, oT_sb)
                        if it[0] >= 2:
                            normed1[0] += 1
            # ---- output projection: r[t, m] = sum_h oT_h.T @ wo_h for
            # the blocks not already streamed into attn-1.  A few blocks
            # (whose rows are long normalized) are emitted BEFORE the last
            # row's denominator flush so the PE queue never idles behind
            # the attention tail's DVE add chain ----
            op_state = {"ridx": 0, "obanks": []}

            def emit_final_block():
                tb, mc = op_blocks[op_i[0]]
                op_i[0] += 1
                wo_mc = wo_mcs[mc]
                ridx = op_state["ridx"]
                if ridx % 8 == 0:
                    op_state["obanks"] = [
                        psum.tile([P, 2, NQ], F32, tag=t, name="pr")
                        for t in ("B45", "B67", "B01", "B23")]
                ps_r = op_state["obanks"][ridx % 4][:, (ridx // 4) % 2, :]
                for h4 in range(HPG):
                    nc.tensor.matmul(
                        ps_r,
                        oT_sb[:, h4 * T + tb * P:h4 * T + (tb + 1) * P],
                        wo_mc[:, h4, :],
                        start=(h4 == 0), stop=(h4 == HPG - 1))
                ro = rout.tile([P, NQ], BF16, tag="ro", name="ro")
                nc.scalar.activation(ro[:], ps_r, AFT.Copy)
                # sync only: the gpsimd queue runs drain/semaphore-clear
                # rounds in this phase and stalls DMAs queued behind them
                nc.sync.dma_start(
                    out=r_d[tb * P:(tb + 1) * P, mc * NQ:(mc + 1) * NQ],
                    in_=ro[:])
                op_state["ridx"] += 1

            pre = 6
            while op_i[0] < len(op_blocks) and pre > 0:
                emit_final_block()
                pre -= 1
            while pend_den:
                flush_den(pend_norm)
            while pend_norm:
                _emit_norm(nc, rtmp, pend_norm.pop(0), oT_sb)
            while op_i[0] < len(op_blocks):
                emit_final_block()

    nc.compile()
    return nc


def _emit_rope_tail(nc, rtmp, tt_sb, tsl, phase2, no_swap=False):
    import concourse.mybir as mybir

    F32 = mybir.dt.float32
    BF16 = mybir.dt.bfloat16
    AFT = mybir.ActivationFunctionType
    for pab, dst in phase2:
        pcd = rtmp.tile([P, NQ], BF16, tag="pcd", name="pcd", bufs=2)
        if no_swap:
            # read the swapped halves of pab directly in two DVE muls —
            # no ACT copy, so attention-phase exps are not pushed back
            nc.vector.tensor_mul(pcd[0:64, :], pab[64:128, :],
                                 tt_sb[64:128, tsl])
            nc.vector.tensor_mul(pcd[64:128, :], pab[0:64, :],
                                 tt_sb[0:64, tsl])
        else:
            sh = rtmp.tile([P, NQ], BF16, tag="sh", name="sh", bufs=2)
            nc.scalar.activation(sh[0:64, :], pab[64:128, :], AFT.Copy)
            nc.scalar.activation(sh[64:128, :], pab[0:64, :], AFT.Copy)
            nc.vector.tensor_mul(pcd[:], sh[:], tt_sb[:, tsl])
        nc.vector.tensor_sub(dst[0:64, tsl], pab[0:64, :], pcd[0:64, :])
        nc.vector.tensor_add(dst[64:128, tsl], pcd[64:128, :],
                             pab[64:128, :])


def _emit_pv(nc, item, ps_oT, v_sb):
    j, q0, pT2, st, sp = item
    for hl in range(2):
        nc.tensor.matmul(
            ps_oT[hl][:, q0:NQ],
            v_sb[:, j * 256 + hl * P:j * 256 + hl * P + P],
            pT2[:, hl, q0:NQ], start=st, stop=sp)


def _emit_norm(nc, rtmp, item, oT_sb):
    import concourse.mybir as mybir

    F32 = mybir.dt.float32
    h4, qc, denc, oc = item
    rec = rtmp.tile([P, NQ], F32, tag="rec", name="rec", bufs=2)
    nc.vector.reciprocal_approx_fast(rec[:], denc[:])
    nc.gpsimd.tensor_mul(
        oT_sb[:, h4 * T + qc * NQ:h4 * T + (qc + 1) * NQ], oc[:], rec[:])


def _host_constants():
    import ml_dtypes

    BF = ml_dtypes.bfloat16
    half = D // 2
    pos = np.arange(T, dtype=np.float64)[:, None]
    freqs = np.power(
        np.float64(ROTARY_BASE),
        -np.arange(half, dtype=np.float64) / np.float64(half))[None, :]
    rad = pos * freqs                               # [T, 64]
    # cos table carries the muP attention scale and the fp8 descale for
    # the q/k projections (x scaled by X8S, weights by W8S)
    desc = np.sqrt(ATTN_SCALE) / np.float64(X8S * W8S)
    cos = np.cos(rad).T * desc                      # [64, T]
    tan = np.tan(rad).T                             # [64, T] = sin/cos
    cc = np.concatenate([cos, cos], axis=0).astype(BF)
    tt = np.concatenate([tan, tan], axis=0).astype(BF)

    kk = np.arange(P)[:, None]
    qq = np.arange(P)[None, :]
    tri = np.where(kk <= qq, 0.0, -1e30).astype(BF)  # [128, 128]
    ones = np.ones((P, P), dtype=BF)
    ident = np.eye(P, dtype=np.float32).astype(BF)
    return cc, tt, tri, ones, ident


def kernel(x, wq, wk, wv, wo):
    import ml_dtypes

    BF = ml_dtypes.bfloat16

    x = np.asarray(x, dtype=np.float32)
    wq = np.asarray(wq, dtype=np.float32)
    wk = np.asarray(wk, dtype=np.float32)
    wv = np.asarray(wv, dtype=np.float32)
    wo = np.asarray(wo, dtype=np.float32)

    from concourse.bass_utils import run_bass_kernel_spmd

    if "nc" not in _CACHE:
        _CACHE["nc"] = _build_program()
    nc = _CACHE["nc"]

    cc, tt, tri, ones, ident = _host_constants()
    F8NP = ml_dtypes.float8_e4m3fn

    def w_layout(w, g):
        # w: [M, H, D] -> group slice -> [P, 2, MB, 256] bf16
        ws = w[:, g * HPG:(g + 1) * HPG, :].astype(np.float32)
        ws = ws.reshape(M, 2, 256)                    # pair-major head axis
        ws = ws.reshape(MB, P, 2, 256).transpose(1, 2, 0, 3)
        return np.ascontiguousarray(ws).astype(BF)

    def w8_layout(w, g):
        # w: [M, H, D] -> [P, pair, dp-block, i, 256] fp8 with contraction
        # pairs (m, m+128) along i for DoubleRow
        ws = (w[:, g * HPG:(g + 1) * HPG, :] * W8S).astype(np.float32)
        ws = ws.reshape(M, 2, 256)                    # pair-major head axis
        ws = ws.reshape(NDP, 2, P, 2, 256).transpose(2, 3, 0, 1, 4)
        ws = np.clip(ws, -240.0, 240.0)
        return np.ascontiguousarray(ws).astype(F8NP)

    xts = [np.ascontiguousarray(x[b].T).astype(BF) for b in range(B)]
    xt8s = [np.clip(np.ascontiguousarray(x[b].T) * X8S,
                    -240.0, 240.0).astype(F8NP) for b in range(B)]
    in_maps = []
    for c in range(N_CORES):
        b, g = divmod(c, GROUPS)
        wog = np.ascontiguousarray(
            wo[g * HPG:(g + 1) * HPG].transpose(1, 0, 2)).astype(BF)
        in_maps.append({
            "xt": xts[b],
            "xt8": xt8s[b],
            "wq": w8_layout(wq, g),
            "wk": w8_layout(wk, g),
            "wv": w_layout(wv, g),
            "wo": wog,
            "trig_cc": cc,
            "trig_tt": tt,
            "tri_neg": tri,
            "onesw": ones,
            "identw": ident,
        })

    res = run_bass_kernel_spmd(nc, in_maps, list(range(N_CORES)))

    r = np.zeros((B, T, M), dtype=np.float32)
    for c in range(N_CORES):
        b = c // GROUPS
        r[b] += np.asarray(res.results[c]["r_out"], dtype=np.float32)
    return r

